# revision 1
# baseline (speedup 1.0000x reference)
"""Trainium2 Bass kernel for nn_PolicyNetwork3 (2-layer GraphSAGE + edge-MLP).

Design (8 NeuronCores, SPMD single NEFF):
- dst-sharded aggregation: core k owns node block [6272k, 6272k+6272).
- Edges sorted by (core, dst-window, src-half); gathered from HBM row tables
  via dma_gather (int16 idx -> LO/HI table halves of 25088 rows).
- segment-sum per 128-dst window via one-hot matmuls accumulating in PSUM
  (one-hot weighted by 1/max(deg,1), built on DVE with tensor_scalar).
- BN folded into the SAGE weights on host; leaky-relu on DVE (mult+max).
- h shards exchanged between layers with an AllGather collective; candidate
  MLP folds its first layer into per-node g/q tables (g=h2@A, q=h2@B+b),
  gathers g[u], q[v] per candidate, finishes the MLP on PE, and computes the
  global softmax on-device after an AllGather of y.
"""

import sys

sys.path.insert(0, "/opt/trn_rl_repo")
sys.path.insert(0, "/root/.axon_site")

import numpy as np

import concourse.bacc as bacc
import concourse.bass as bass
import concourse.bass_isa as bass_isa
import concourse.mybir as mybir
import concourse.tile as tile
from concourse import library_config
from concourse.bass_utils import run_bass_kernel_spmd

P = 128
N, E, C = 50000, 800000, 100000
D = 128
NCORE = 8
NSH = 6272          # nodes per core shard
NTOT = NSH * NCORE  # 50176 padded node table
NWIN = NSH // 64    # 98 aggregation windows (64 dst) per core
NBLK = NSH // P     # 49 node blocks for the linear phase
HALF = NTOT // 2    # 25088 rows per gather-table half
CSH = C // NCORE    # 12500 candidates per core
GCALL = 2048        # max idxs per dma_gather call
BN_EPS = 1e-5
SLOPE = 0.01
F32 = mybir.dt.float32
I16 = mybir.dt.int16
AF = mybir.ActivationFunctionType
ALU = mybir.AluOpType


def _wrap16(idx_lin):
    """[n] -> [128, n/16] int16 in the dma_gather wrapped+replicated layout."""
    n = idx_lin.shape[0]
    assert n % 16 == 0
    w = idx_lin.reshape(n // 16, 16).T.astype(np.int16)
    return np.tile(w, (8, 1)).copy()


def _slotize(vals, nslot, fill=0.0, dtype=np.float32):
    """[n] values -> [128, nslot/128] laid out so slot i = [i%128, i//128]."""
    out = np.full(nslot, fill, dtype)
    out[: vals.shape[0]] = vals
    return out.reshape(nslot // P, P).T.copy()


def _prep_edges(src, dst, invdeg):
    """Build the uniform per-core chunk schedule + per-core index data."""
    core = np.minimum(dst // NSH, NCORE - 1)
    winl = (dst - core * NSH) // 64
    half = (src >= HALF).astype(np.int64)
    key = (core * NWIN + winl) * 2 + half
    order = np.argsort(key, kind="stable")
    ksort = key[order]
    cnt = np.bincount(key, minlength=NCORE * NWIN * 2).reshape(NCORE, NWIN * 2)
    nch = -(-cnt // P)                       # ceil chunks per (core, win*2+half)
    nch_u = nch.max(axis=0)                  # [NWIN*2] uniform chunk counts
    # stream order: all LO runs (win 0..48), then all HI runs
    runs = [(w, s) for s in (0, 1) for w in range(NWIN)]
    run_nch = [int(nch_u[w * 2 + s]) for (w, s) in runs]
    tot_ch = sum(run_nch)
    nslot = tot_ch * P

    gidx = np.zeros((NCORE, nslot), np.int16)
    dstloc = np.full((NCORE, nslot), -5.0, np.float32)
    val = np.zeros((NCORE, nslot), np.float32)
    # per-core bucket start offsets in the sorted edge array
    bstart = np.zeros(NCORE * NWIN * 2 + 1, np.int64)
    np.cumsum(np.bincount(key, minlength=NCORE * NWIN * 2), out=bstart[1:])
    for k in range(NCORE):
        pos = 0
        for s in (0, 1):
            for w in range(NWIN):
                b = (k * NWIN + w) * 2 + s
                e0, e1 = bstart[b], bstart[b + 1]
                n = e1 - e0
                sl = order[e0:e1]
                sl = sl[np.argsort(src[sl], kind="stable")]
                gidx[k, pos : pos + n] = (src[sl] - s * HALF).astype(np.int16)
                dstloc[k, pos : pos + n] = (dst[sl] % 64).astype(np.float32)
                val[k, pos : pos + n] = invdeg[dst[sl]]
                pos += nch_u[w * 2 + s] * P
    # gather call split: contiguous LO slots then HI slots, calls <= GCALL
    n_lo = sum(run_nch[:NWIN]) * P
    calls = []  # (slot_start, n_idx, half)
    for s, lo, hi in ((0, 0, n_lo), (1, n_lo, nslot)):
        p = lo
        while p < hi:
            n = min(GCALL, hi - p)
            calls.append((p, n, s))
            p += n
    # per-run chunk offsets
    run_off = np.zeros(len(runs) + 1, np.int64)
    np.cumsum(run_nch, out=run_off[1:])
    meta = dict(runs=runs, run_nch=run_nch, run_off=run_off, tot_ch=tot_ch,
                nslot=nslot, calls=calls)
    data = [dict(gidx=_wrap16(gidx[k]),
                 dstloc=gidx_to_cols(dstloc[k]),
                 val=gidx_to_cols(val[k])) for k in range(NCORE)]
    return meta, data


def gidx_to_cols(arr):
    """[nslot] -> [128, nchunk] with slot i at [i%128, i//128]."""
    n = arr.shape[0]
    return arr.reshape(n // P, P).T.copy()


def _prep_cands(cand_u, cand_v, cand_feat):
    """Shard candidates, group by (u_half, v_half), pad to uniform chunks."""
    percore = [np.arange(k * CSH, (k + 1) * CSH) for k in range(NCORE)]
    groups = [[None] * 4 for _ in range(NCORE)]
    for k in range(NCORE):
        ids = percore[k]
        g = (cand_u[ids] >= HALF) * 2 + (cand_v[ids] >= HALF)
        o = np.argsort(g, kind="stable")
        ids = ids[o]
        gs = g[o]
        for gi in range(4):
            gids = ids[gs == gi]
            groups[k][gi] = gids[np.argsort(cand_u[gids], kind="stable")]
    gch = np.zeros((NCORE, 4), np.int64)
    for k in range(NCORE):
        for gi in range(4):
            gch[k, gi] = -(-len(groups[k][gi]) // P)
    gch_u = gch.max(axis=0)                 # uniform chunks per group
    ncc = int(gch_u.sum())
    cslot = ncc * P
    cu = np.zeros((NCORE, cslot), np.int16)
    cv = np.zeros((NCORE, cslot), np.int16)
    ft = np.zeros((NCORE, cslot), np.float32)
    mask = np.full((NCORE, cslot), -1e30, np.float32)
    slotmap = np.full((NCORE, cslot), -1, np.int64)
    goff = np.zeros(5, np.int64)
    np.cumsum(gch_u * P, out=goff[1:])
    for k in range(NCORE):
        for gi in range(4):
            ids = groups[k][gi]
            n = len(ids)
            p0 = goff[gi]
            uh, vh = gi // 2, gi % 2
            cu[k, p0 : p0 + n] = (cand_u[ids] - uh * HALF).astype(np.int16)
            cv[k, p0 : p0 + n] = (cand_v[ids] - vh * HALF).astype(np.int16)
            ft[k, p0 : p0 + n] = cand_feat[ids, 0]
            mask[k, p0 : p0 + n] = 0.0
            slotmap[k, p0 : p0 + n] = ids
    # gather calls: u -> runs (groups 0-1 | 2-3); v -> one run per group
    ucalls, vcalls = [], []
    for s, lo, hi in ((0, goff[0], goff[2]), (1, goff[2], goff[4])):
        p = lo
        while p < hi:
            n = min(GCALL, hi - p)
            ucalls.append((int(p), int(n), s))
            p += n
    for gi in range(4):
        p, hi = goff[gi], goff[gi + 1]
        while p < hi:
            n = min(GCALL, hi - p)
            vcalls.append((int(p), int(n), gi % 2))
            p += n
    meta = dict(ncc=ncc, cslot=cslot, ucalls=ucalls, vcalls=vcalls)
    data = [dict(cu=_wrap16(cu[k]), cv=_wrap16(cv[k]),
                 feat=gidx_to_cols(ft[k]), mask=gidx_to_cols(mask[k]),
                 slotmap=slotmap[k]) for k in range(NCORE)]
    return meta, data


def _build_nc(em, cm):
    nc = bacc.Bacc("TRN2", target_bir_lowering=False, debug=False,
                   num_devices=NCORE)
    f32 = F32
    TOTCH, NSLOT = em["tot_ch"], em["nslot"]
    NCC, CSLOT = cm["ncc"], cm["cslot"]

    # ---- external inputs ----
    xpad = nc.dram_tensor("xpad", [NTOT, D], f32, kind="ExternalInput")
    xT = nc.dram_tensor("xT", [P, NSH], f32, kind="ExternalInput")
    gidx = nc.dram_tensor("gidx", [P, NSLOT // 16], I16, kind="ExternalInput")
    dstloc = nc.dram_tensor("dstloc", [P, TOTCH], f32, kind="ExternalInput")
    val = nc.dram_tensor("val", [P, TOTCH], f32, kind="ExternalInput")
    wself = [nc.dram_tensor(f"wself{l}", [D, D], f32, kind="ExternalInput") for l in range(2)]
    wneigh = [nc.dram_tensor(f"wneigh{l}", [D, D], f32, kind="ExternalInput") for l in range(2)]
    crow = [nc.dram_tensor(f"crow{l}", [1, D], f32, kind="ExternalInput") for l in range(2)]
    iota = nc.dram_tensor("iota", [P, 64], f32, kind="ExternalInput")
    ident = nc.dram_tensor("ident", [P, P], f32, kind="ExternalInput")
    onesr = nc.dram_tensor("onesr", [1, P], f32, kind="ExternalInput")
    amat = nc.dram_tensor("amat", [D, 64], f32, kind="ExternalInput")
    bmat = nc.dram_tensor("bmat", [D, 64], f32, kind="ExternalInput")
    mw0r = nc.dram_tensor("mw0r", [P, 64], f32, kind="ExternalInput")
    mb0r = nc.dram_tensor("mb0r", [1, 64], f32, kind="ExternalInput")
    mw1 = nc.dram_tensor("mw1", [64, 64], f32, kind="ExternalInput")
    mb1c = nc.dram_tensor("mb1c", [64, 1], f32, kind="ExternalInput")
    mw2 = nc.dram_tensor("mw2", [64, 1], f32, kind="ExternalInput")
    mb2v = nc.dram_tensor("mb2v", [P, 1], f32, kind="ExternalInput")
    cu = nc.dram_tensor("cu", [P, CSLOT // 16], I16, kind="ExternalInput")
    cv = nc.dram_tensor("cv", [P, CSLOT // 16], I16, kind="ExternalInput")
    feat = nc.dram_tensor("feat", [P, NCC], f32, kind="ExternalInput")
    maskr = nc.dram_tensor("maskr", [P, NCC], f32, kind="ExternalInput")
    # ---- outputs ----
    y_out = nc.dram_tensor("y_out", [P, NCC], f32, kind="ExternalOutput")
    p_out = nc.dram_tensor("p_out", [P, NCORE * CSLOT // P], f32, kind="ExternalOutput")
    # ---- internal DRAM ----
    hsh = [nc.dram_tensor(f"hsh{l}", [NSH, D], f32, kind="Internal") for l in range(2)]
    hfull = nc.dram_tensor("hfull", [NTOT, D], f32, kind="Internal", addr_space="Shared")
    gqsh = nc.dram_tensor("gqsh", [NSH, D], f32, kind="Internal")
    gqfull = nc.dram_tensor("gqfull", [NTOT, D], f32, kind="Internal", addr_space="Shared")
    ysh = nc.dram_tensor("ysh", [P, NCC], f32, kind="Internal")
    yfull = nc.dram_tensor("yfull", [NCORE * P, NCC], f32, kind="Internal", addr_space="Shared")

    rg = [list(range(NCORE))]

    with tile.TileContext(nc) as tc:
        with (
            tc.tile_pool(name="const", bufs=1) as cp,
            tc.tile_pool(name="big", bufs=1) as bp,
            tc.tile_pool(name="msgs", bufs=3) as mp,
            tc.tile_pool(name="oh", bufs=6) as ohp,
            tc.tile_pool(name="wrk", bufs=4) as wp,
            tc.tile_pool(name="ps_run", bufs=2, space="PSUM") as ps_run,
            tc.tile_pool(name="ps_t", bufs=2, space="PSUM") as ps_t,
            tc.tile_pool(name="ps_h", bufs=2, space="PSUM") as ps_h,
            tc.tile_pool(name="ps_s", bufs=2, space="PSUM") as ps_s,
        ):
            nc.gpsimd.load_library(library_config.mlp)

            def load(pool, t, shape=None):
                tl = pool.tile(shape or list(t.shape), t.dtype, tag=t.name)
                nc.sync.dma_start(tl[:], t[:])
                return tl

            gidx_t = load(cp, gidx)
            dstloc_t = load(cp, dstloc)
            val_t = load(cp, val)
            iota_t = load(cp, iota)
            ident_t = load(cp, ident)
            onesr_t = load(cp, onesr)
            wself_t = [load(cp, w) for w in wself]
            wneigh_t = [load(cp, w) for w in wneigh]
            crow_t = [load(cp, w) for w in crow]
            amat_t = load(cp, amat)
            bmat_t = load(cp, bmat)
            mw0r_t = load(cp, mw0r)
            mb0r_t = load(cp, mb0r)
            mw1_t = load(cp, mw1)
            mb1c_t = load(cp, mb1c)
            mw2_t = load(cp, mw2)
            mb2_t = load(cp, mb2v)
            cu_t = load(cp, cu)
            cv_t = load(cp, cv)
            feat_t = load(cp, feat)
            mask_t = load(cp, maskr)

            curT = bp.tile([P, NSH], f32, tag="curT")
            nxtT = bp.tile([P, NSH], f32, tag="nxtT")
            aggr = bp.tile([P, NSH], f32, tag="aggr")
            nc.sync.dma_start(curT[:], xT[:])

            runs, run_nch, run_off = em["runs"], em["run_nch"], em["run_off"]
            # chunk -> (run_index, pos_in_run)
            ch_run = []
            for ri, nchk in enumerate(run_nch):
                ch_run += [(ri, j, nchk) for j in range(nchk)]

            for layer in range(2):
                gtab = xpad if layer == 0 else hfull
                ps = None
                for (s0, n_idx, s) in em["calls"]:
                    c0, ncall = s0 // P, n_idx // P
                    g = mp.tile([P, GCALL // P, P], f32, tag="g")
                    nc.gpsimd.dma_gather(
                        g[:, :ncall, :],
                        gtab[s * HALF : (s + 1) * HALF, :],
                        gidx_t[:, s0 // 16 : (s0 + n_idx) // 16],
                        n_idx, n_idx, P, single_packet=False)
                    for cc in range(ncall):
                        ch = c0 + cc
                        ri, j, nchk = ch_run[ch]
                        w, sh = runs[ri]
                        oh = ohp.tile([P, 64], f32, tag="oh")
                        nc.vector.tensor_scalar(
                            oh[:], iota_t[:], dstloc_t[:, ch : ch + 1],
                            val_t[:, ch : ch + 1], ALU.is_equal, ALU.mult)
                        if j == 0:
                            ps = ps_run.tile([P, 64], f32, tag="psw")
                        nc.tensor.matmul(ps[:], lhsT=g[:, cc, :], rhs=oh[:],
                                         start=(j == 0), stop=(j == nchk - 1))
                        if j == nchk - 1:
                            wsl = aggr[:, w * 64 : (w + 1) * 64]
                            lo_empty = run_nch[runs.index((w, 0))] == 0
                            if sh == 0 or lo_empty:
                                nc.scalar.activation(wsl, ps[:], AF.Copy)
                            else:
                                nc.vector.tensor_tensor(wsl, wsl, ps[:], ALU.add)
                # windows with zero chunks in both passes
                for w in range(NWIN):
                    if (run_nch[runs.index((w, 0))] == 0
                            and run_nch[runs.index((w, 1))] == 0):
                        nc.vector.memset(aggr[:, w * 64 : (w + 1) * 64], 0.0)
                # per-window linear + BN + leaky
                for w in range(NBLK):
                    ph = ps_h.tile([P, P], f32, tag="ph")
                    nc.tensor.matmul(ph[:], lhsT=aggr[:, w * P : (w + 1) * P],
                                     rhs=wneigh_t[layer][:],
                                     start=True, stop=False)
                    nc.tensor.matmul(ph[:], lhsT=curT[:, w * P : (w + 1) * P],
                                     rhs=wself_t[layer][:], start=False, stop=False)
                    nc.tensor.matmul(ph[:], lhsT=onesr_t[:], rhs=crow_t[layer][:],
                                     start=False, stop=True)
                    tmp = wp.tile([P, P], f32, tag="tmp")
                    nc.vector.tensor_scalar(tmp[:], ph[:], SLOPE, None, ALU.mult)
                    ht = wp.tile([P, P], f32, tag="ht")
                    nc.vector.tensor_tensor(ht[:], ph[:], tmp[:], ALU.max)
                    nc.sync.dma_start(hsh[layer][w * P : (w + 1) * P, :], ht[:])
                    pt2 = ps_t.tile([P, P], f32, tag="pt")
                    nc.tensor.transpose(pt2[:], ht[:], ident_t[:])
                    nc.scalar.activation(nxtT[:, w * P : (w + 1) * P], pt2[:], AF.Copy)
                if layer == 0:
                    nc.gpsimd.collective_compute(
                        "AllGather", ALU.bypass, replica_groups=rg,
                        ins=[hsh[0][:].opt()], outs=[hfull[:].opt()])
                curT, nxtT = nxtT, curT

            # ---- g/q tables ----
            for w in range(NBLK):
                pg = ps_h.tile([P, 64], f32, tag="ph")
                nc.tensor.matmul(pg[:], lhsT=curT[:, w * P : (w + 1) * P],
                                 rhs=amat_t[:], start=True, stop=True)
                gq = wp.tile([P, P], f32, tag="gq")
                nc.scalar.activation(gq[:, 0:64], pg[:], AF.Copy)
                pq = ps_h.tile([P, 64], f32, tag="ph")
                nc.tensor.matmul(pq[:], lhsT=curT[:, w * P : (w + 1) * P],
                                 rhs=bmat_t[:], start=True, stop=False)
                nc.tensor.matmul(pq[:], lhsT=onesr_t[:], rhs=mb0r_t[:],
                                 start=False, stop=True)
                nc.scalar.activation(gq[:, 64:128], pq[:], AF.Copy)
                nc.sync.dma_start(gqsh[w * P : (w + 1) * P, :], gq[:])
            nc.gpsimd.collective_compute(
                "AllGather", ALU.bypass, replica_groups=rg,
                ins=[gqsh[:].opt()], outs=[gqfull[:].opt()])

            # ---- candidate gathers ----
            ut = bp.tile([P, NCC, 64], f32, tag="aggr")
            vt = bp.tile([P, NCC, 64], f32, tag="curT")
            for (tl, idx_t, calls, cofs) in ((ut, cu_t, cm["ucalls"], 0),
                                             (vt, cv_t, cm["vcalls"], 64)):
                for (s0, n_idx, s) in calls:
                    nc.gpsimd.dma_gather(
                        tl[:, s0 // P : (s0 + n_idx) // P, :],
                        gqfull[s * HALF : (s + 1) * HALF, cofs : cofs + 64],
                        idx_t[:, s0 // 16 : (s0 + n_idx) // 16],
                        n_idx, n_idx, 64, elem_step=P, single_packet=False)
            # ---- candidate MLP ----
            z1 = bp.tile([P, NCC, 64], f32, tag="nxtT")
            for c in range(NCC):
                nc.vector.tensor_scalar(z1[:, c, :], mw0r_t[:],
                                        feat_t[:, c : c + 1], None, ALU.mult)
            nc.vector.tensor_tensor(z1[:], z1[:], ut[:], ALU.add)
            nc.vector.tensor_tensor(z1[:], z1[:], vt[:], ALU.add)
            zt = bp.tile([P, NCC, 64], f32, tag="aggr")
            nc.vector.tensor_scalar(zt[:], z1[:], SLOPE, None, ALU.mult)
            nc.vector.tensor_tensor(z1[:], z1[:], zt[:], ALU.max)
            ycol = wp.tile([P, NCC], f32, tag="ycol")
            for c in range(NCC):
                pzt = ps_t.tile([64, P], f32, tag="pt")
                nc.tensor.transpose(pzt[:], z1[:, c, :], ident_t[:])
                z1t = wp.tile([64, P], f32, tag="z1t")
                nc.scalar.activation(z1t[:], pzt[:], AF.Copy)
                pz = ps_s.tile([64, P], f32, tag="pz")
                nc.tensor.matmul(pz[:], lhsT=mw1_t[:], rhs=z1t[:],
                                 start=True, stop=True)
                zb = wp.tile([64, P], f32, tag="zb")
                nc.vector.tensor_scalar(zb[:], pz[:], mb1c_t[:, 0:1], None, ALU.add)
                zs = wp.tile([64, P], f32, tag="zs")
                nc.vector.tensor_scalar(zs[:], zb[:], SLOPE, None, ALU.mult)
                z2 = wp.tile([64, P], f32, tag="z2")
                nc.vector.tensor_tensor(z2[:], zb[:], zs[:], ALU.max)
                py = ps_s.tile([P, 1], f32, tag="pz")
                nc.tensor.matmul(py[:], lhsT=z2[:], rhs=mw2_t[:],
                                 start=True, stop=True)
                nc.vector.tensor_scalar(ycol[:, c : c + 1], py[:],
                                        mb2_t[:, 0:1], None, ALU.add)
            nc.sync.dma_start(y_out[:], ycol[:])
            ym = wp.tile([P, NCC], f32, tag="ym")
            nc.vector.tensor_tensor(ym[:], ycol[:], mask_t[:], ALU.add)
            nc.sync.dma_start(ysh[:], ym[:])
            nc.gpsimd.collective_compute(
                "AllGather", ALU.bypass, replica_groups=rg,
                ins=[ysh[:].opt()], outs=[yfull[:].opt()])
            # ---- softmax ----
            ncols = NCORE * CSLOT // P
            yf = bp.tile([P, ncols], f32, tag="yf")
            nc.sync.dma_start(yf[:], yfull[:].rearrange("a b -> (a b)")
                              .rearrange("(p c) -> p c", p=P))
            rmax = wp.tile([P, 1], f32, tag="rmax")
            nc.vector.tensor_reduce(rmax[:], yf[:], mybir.AxisListType.X, ALU.max)
            gmax = wp.tile([P, 1], f32, tag="gmax")
            nc.gpsimd.partition_all_reduce(gmax[:], rmax[:], P,
                                           bass_isa.ReduceOp.max)
            ngmax = wp.tile([P, 1], f32, tag="ngmax")
            nc.vector.tensor_scalar(ngmax[:], gmax[:], -1.0, None, ALU.mult)
            ef = bp.tile([P, ncols], f32, tag="ef")
            se = wp.tile([P, 1], f32, tag="se")
            nc.scalar.activation(ef[:], yf[:], AF.Exp, bias=ngmax[:, 0:1],
                                 accum_out=se[:])
            stot = wp.tile([P, 1], f32, tag="stot")
            nc.gpsimd.partition_all_reduce(stot[:], se[:], P, bass_isa.ReduceOp.add)
            invs = wp.tile([P, 1], f32, tag="invs")
            nc.vector.reciprocal(invs[:], stot[:])
            pf = bp.tile([P, ncols], f32, tag="pf")
            nc.vector.tensor_scalar(pf[:], ef[:], invs[:, 0:1], None, ALU.mult)
            nc.sync.dma_start(p_out[:], pf[:])
    nc.compile()
    return nc


def kernel(x, src, dst, cand_u, cand_v, cand_feat,
           w_self0, w_neigh0, b0, gamma0, beta0, rm0, rv0,
           w_self1, w_neigh1, b1, gamma1, beta1, rm1, rv1,
           mw0, mb0, mw1, mb1, mw2, mb2):
    x = np.asarray(x, np.float32)
    src = np.asarray(src, np.int64)
    dst = np.asarray(dst, np.int64)
    cand_u = np.asarray(cand_u, np.int64)
    cand_v = np.asarray(cand_v, np.int64)
    cand_feat = np.asarray(cand_feat, np.float32)

    deg = np.bincount(dst, minlength=N).astype(np.float32)
    invdeg = 1.0 / np.maximum(deg, 1.0)
    em, edata = _prep_edges(src, dst, invdeg)
    cm, cdata = _prep_cands(cand_u, cand_v, cand_feat)

    xpad = np.zeros((NTOT, D), np.float32)
    xpad[:N] = x
    iota = np.tile(np.arange(64, dtype=np.float32), (P, 1))
    ident = np.eye(P, dtype=np.float32)
    onesr = np.ones((1, P), np.float32)

    com = {"xpad": xpad, "iota": iota, "ident": ident, "onesr": onesr}
    for l, (ws, wn, b, ga, be, rme, rve) in enumerate(
        ((w_self0, w_neigh0, b0, gamma0, beta0, rm0, rv0),
         (w_self1, w_neigh1, b1, gamma1, beta1, rm1, rv1))):
        a = (ga / np.sqrt(rve + BN_EPS)).astype(np.float32)
        com[f"wself{l}"] = (ws * a[None, :]).astype(np.float32)
        com[f"wneigh{l}"] = (wn * a[None, :]).astype(np.float32)
        com[f"crow{l}"] = (a * (b - rme) + be).astype(np.float32)[None, :]
    com["amat"] = np.asarray(mw0[0:128], np.float32)
    com["bmat"] = np.asarray(mw0[128:256], np.float32)
    com["mw0r"] = np.tile(np.asarray(mw0[256], np.float32), (P, 1))
    com["mb0r"] = np.asarray(mb0, np.float32)[None, :]
    com["mw1"] = np.asarray(mw1, np.float32)
    com["mb1c"] = np.asarray(mb1, np.float32)[:, None]
    com["mw2"] = np.asarray(mw2, np.float32)
    com["mb2v"] = np.full((P, 1), np.float32(np.asarray(mb2).ravel()[0]))

    nc = _build_nc(em, cm)
    in_maps = []
    for k in range(NCORE):
        m = dict(com)
        m["xT"] = xpad[k * NSH : (k + 1) * NSH].T.copy()
        m["gidx"] = edata[k]["gidx"]
        m["dstloc"] = edata[k]["dstloc"]
        m["val"] = edata[k]["val"]
        m["cu"] = cdata[k]["cu"]
        m["cv"] = cdata[k]["cv"]
        m["feat"] = cdata[k]["feat"]
        m["maskr"] = cdata[k]["mask"]
        in_maps.append(m)
    import os
    trace = bool(os.environ.get("KERNEL_TRACE"))
    if trace:
        import types
        import ctypes
        if "antenv.axon_hooks" not in sys.modules:
            try:
                import antenv
                from trn_agent_boot.trn_boot import _ntff_profile_via_ctypes
                mod = types.ModuleType("antenv.axon_hooks")
                hook = [_ntff_profile_via_ctypes("/opt/axon/libaxon_pjrt.so")]
                mod.set_axon_ntff_profile_hook = lambda h: hook.__setitem__(0, h)
                mod.get_axon_ntff_profile_hook = lambda: hook[0]
                sys.modules["antenv.axon_hooks"] = mod
                antenv.axon_hooks = mod
            except Exception:
                trace = False
    res = run_bass_kernel_spmd(nc, in_maps, core_ids=list(range(NCORE)),
                               trace=trace,
                               tmpdir=os.environ.get("KERNEL_TRACE_DIR"))
    if trace and res.exec_time_ns is not None:
        print(f"HW exec time: {res.exec_time_ns} ns")
    y_all = np.zeros(C, np.float32)
    p_all = np.zeros(C, np.float32)
    ncc = cm["ncc"]
    p_lin = res.results[0]["p_out"].ravel()   # global order: k, p, c
    for k in range(NCORE):
        sm = cdata[k]["slotmap"]
        valid = sm >= 0
        j = np.nonzero(valid)[0]              # slot j = c*128 + p
        yk = res.results[k]["y_out"]          # [128, NCC] -> value at [j%128, j//128]
        y_all[sm[valid]] = yk[j % P, j // P]
        gs = k * cm["cslot"] + (j % P) * ncc + (j // P)
        p_all[sm[valid]] = p_lin[gs]
    return y_all[:, None], p_all[:, None]



# revision 8
# speedup vs baseline: 2.4744x; 2.4744x over previous
"""Trainium2 Bass kernel for nn_PolicyNetwork3 (2-layer GraphSAGE + edge-MLP).

v2 design (8 NeuronCores, SPMD single NEFF):
- dst-sharded aggregation; core k owns node block [6272k, 6272k+6272).
- Edge messages gathered per-edge from bf16 HBM row tables via 4-queue
  round-robin dma_gather (descriptor generation parallelizes across the
  SWDGE queues; ~2.4ns/row vs 8ns serialized).
- segment-sum per 256-dst window via one-hot matmuls (bf16 gathered rows x
  fp8 0/1 one-hot streamed from HBM) accumulating in PSUM across the LO/HI
  half-table runs; per-window PSUM->SBUF copy fuses the 1/deg scale.
- BN folded into SAGE weights on host; linear+leaky per 128-node block;
  h shards exchanged with bf16 AllGather.
- candidate MLP: g/q tables per node; transposed candidate gathers put
  features on partitions so the whole MLP runs as 512-wide PE matmuls
  (identity-matmul adds, K=1 bias rows); global softmax on-device.
"""

import os
import sys

sys.path.insert(0, "/opt/trn_rl_repo")
sys.path.insert(0, "/root/.axon_site")

import numpy as np
import ml_dtypes

import concourse.bacc as bacc
import concourse.bass as bass
import concourse.bass_isa as bass_isa
import concourse.mybir as mybir
import concourse.tile as tile
from concourse import library_config
from concourse.bass_utils import run_bass_kernel_spmd

P = 128
N, E, C = 50000, 800000, 100000
D = 128
NCORE = 8
NSH = 6272            # nodes per core shard
NTOT = NSH * NCORE    # 50176 padded node table
HALF = NTOT // 2      # 25088 rows per gather-table half
W = 256               # dst window width (PSUM cols)
NWIN = (NSH + W - 1) // W   # 25 windows (last one 128 wide)
NBLK = NSH // P       # 49 node blocks for the linear phase
CSH = C // NCORE      # 12500 candidates per core
GCALL = 2048          # max idxs per dma_gather call
CCH = 512             # candidate MLP chunk
BN_EPS = 1e-5
SLOPE = 0.01
F32 = mybir.dt.float32
BF16 = mybir.dt.bfloat16
F8 = mybir.dt.float8e4
I16 = mybir.dt.int16
AF = mybir.ActivationFunctionType
ALU = mybir.AluOpType
BF = ml_dtypes.bfloat16
F8NP = ml_dtypes.float8_e4m3fn


def _wrap16(idx_lin):
    """[n] -> [128, n/16] int16 in the dma_gather wrapped+replicated layout."""
    n = idx_lin.shape[0]
    assert n % 16 == 0
    w = idx_lin.reshape(n // 16, 16).T.astype(np.int16)
    return np.tile(w, (8, 1)).copy()


def _win_width(w):
    return min(W, NSH - w * W)


def _prep_edges(src, dst):
    """Uniform per-core window/run/chunk schedule + per-core idx and one-hot."""
    core = dst // NSH
    winl = (dst - core * NSH) // W
    dstloc = (dst - core * NSH) - winl * W
    half = (src >= HALF).astype(np.int64)
    key = (core * NWIN + winl) * 2 + half
    order = np.argsort(key, kind="stable")
    cnt = np.bincount(key, minlength=NCORE * NWIN * 2).reshape(NCORE, NWIN, 2)
    nch_u = (-(-cnt // P)).max(axis=0)        # [NWIN, 2] uniform chunk counts
    # global chunk order: w0 LO chunks, w0 HI, w1 LO, ...
    win_ch0 = np.zeros(NWIN + 1, np.int64)
    np.cumsum(nch_u.sum(axis=1), out=win_ch0[1:])
    totch = int(win_ch0[-1])
    nslot = totch * P
    # gather calls (uniform): per (w, half) run split into <=GCALL
    calls = []  # (slot_start, n_idx, half, chunk0)
    for w in range(NWIN):
        c0 = int(win_ch0[w])
        for s in (0, 1):
            nch = int(nch_u[w, s])
            p0, left = c0 * P, nch * P
            while left > 0:
                n = min(GCALL, left)
                calls.append((p0, n, s, p0 // P))
                p0 += n
                left -= n
            c0 += nch
    bstart = np.zeros(NCORE * NWIN * 2 + 1, np.int64)
    np.cumsum(np.bincount(key, minlength=NCORE * NWIN * 2), out=bstart[1:])
    gidx = np.zeros((NCORE, nslot), np.int16)
    ohm = np.zeros((NCORE, P, totch * W), np.uint8)  # fp8 bits (1.0 = 0x38)
    ONE = np.float32(1.0).astype(F8NP).view(np.uint8)
    for k in range(NCORE):
        for w in range(NWIN):
            pos = int(win_ch0[w]) * P
            for s in (0, 1):
                b = (k * NWIN + w) * 2 + s
                sl = order[bstart[b]:bstart[b + 1]]
                n = len(sl)
                gidx[k, pos:pos + n] = (src[sl] - s * HALF).astype(np.int16)
                slots = pos + np.arange(n)
                ohm[k, slots % P, (slots // P) * W + dstloc[sl]] = ONE
                pos += int(nch_u[w, s]) * P
    meta = dict(nch_u=nch_u, win_ch0=win_ch0, totch=totch, nslot=nslot,
                calls=calls)
    data = [dict(gidx=_wrap16(gidx[k]), ohm=ohm[k].view(F8NP)) for k in range(NCORE)]
    return meta, data


def _prep_cands(cand_u, cand_v, cand_feat):
    """Shard candidates, group by (u_half, v_half), pad groups to 128."""
    gch = np.zeros((NCORE, 4), np.int64)
    groups = [[None] * 4 for _ in range(NCORE)]
    for k in range(NCORE):
        ids = np.arange(k * CSH, (k + 1) * CSH)
        g = (cand_u[ids] >= HALF) * 2 + (cand_v[ids] >= HALF)
        for gi in range(4):
            groups[k][gi] = ids[g == gi]
            gch[k, gi] = -(-len(groups[k][gi]) // P)
    gch_u = gch.max(axis=0)
    goff = np.zeros(5, np.int64)
    np.cumsum(gch_u * P, out=goff[1:])
    cslot = -(-int(goff[4]) // CCH) * CCH      # pad to CCH multiple
    cu = np.zeros((NCORE, cslot), np.int16)
    cv = np.zeros((NCORE, cslot), np.int16)
    ft = np.zeros((NCORE, cslot), BF)
    mask = np.full((NCORE, cslot), -1e30, np.float32).astype(BF)
    slotmap = np.full((NCORE, cslot), -1, np.int64)
    for k in range(NCORE):
        for gi in range(4):
            ids = groups[k][gi]
            n = len(ids)
            p0 = int(goff[gi])
            uh, vh = gi // 2, gi % 2
            cu[k, p0:p0 + n] = (cand_u[ids] - uh * HALF).astype(np.int16)
            cv[k, p0:p0 + n] = (cand_v[ids] - vh * HALF).astype(np.int16)
            ft[k, p0:p0 + n] = cand_feat[ids, 0].astype(BF)
            mask[k, p0:p0 + n] = 0.0
            slotmap[k, p0:p0 + n] = ids
    # u calls: groups 0-1 (uh=0) then 2-3 (uh=1); v calls per group
    ucalls, vcalls = [], []
    for s, lo, hi in ((0, 0, int(goff[2])), (1, int(goff[2]), int(goff[4]))):
        p = lo
        while p < hi:
            n = min(GCALL, hi - p)
            ucalls.append((p, n, s))
            p += n
    for gi in range(4):
        p, hi = int(goff[gi]), int(goff[gi + 1])
        while p < hi:
            n = min(GCALL, hi - p)
            vcalls.append((p, n, gi % 2))
            p += n
    meta = dict(cslot=cslot, ucalls=ucalls, vcalls=vcalls)
    data = [dict(cu=_wrap16(cu[k]), cv=_wrap16(cv[k]), feat=ft[k][None, :],
                 mask=mask[k][None, :], slotmap=slotmap[k]) for k in range(NCORE)]
    return meta, data


def _build_nc(em, cm):
    nc = bacc.Bacc("TRN2", target_bir_lowering=False, debug=False,
                   num_devices=NCORE, num_swdge_queues=4)
    TOTCH, NSLOT = em["totch"], em["nslot"]
    CSLOT = cm["cslot"]
    NCC = CSLOT // CCH                      # candidate MLP chunks
    YCOLS = NCORE * CSLOT // P              # yfull viewed as [128, YCOLS]

    xb = nc.dram_tensor("xb", [NTOT, D], BF16, kind="ExternalInput")
    xT = nc.dram_tensor("xT", [P, NSH], BF16, kind="ExternalInput")
    gidx = nc.dram_tensor("gidx", [P, NSLOT // 16], I16, kind="ExternalInput")
    ohm = nc.dram_tensor("ohm", [P, TOTCH * W], F8, kind="ExternalInput")
    invd = nc.dram_tensor("invd", [P, NSH], BF16, kind="ExternalInput")
    wself = [nc.dram_tensor(f"wself{l}", [D, D], BF16, kind="ExternalInput") for l in range(2)]
    wneigh = [nc.dram_tensor(f"wneigh{l}", [D, D], BF16, kind="ExternalInput") for l in range(2)]
    crow = [nc.dram_tensor(f"crow{l}", [1, D], BF16, kind="ExternalInput") for l in range(2)]
    identb = nc.dram_tensor("identb", [P, P], BF16, kind="ExternalInput")
    onesr = nc.dram_tensor("onesr", [1, P], BF16, kind="ExternalInput")
    onesc = nc.dram_tensor("onesc", [1, CCH], BF16, kind="ExternalInput")
    e1 = nc.dram_tensor("e1", [P, 64], BF16, kind="ExternalInput")
    e2 = nc.dram_tensor("e2", [P, 64], BF16, kind="ExternalInput")
    amat = nc.dram_tensor("amat", [D, 64], BF16, kind="ExternalInput")
    bmat = nc.dram_tensor("bmat", [D, 64], BF16, kind="ExternalInput")
    mw0r = nc.dram_tensor("mw0r", [1, 64], BF16, kind="ExternalInput")
    mb0r = nc.dram_tensor("mb0r", [1, 64], BF16, kind="ExternalInput")
    mw1 = nc.dram_tensor("mw1", [64, 64], BF16, kind="ExternalInput")
    mb1r = nc.dram_tensor("mb1r", [1, 64], BF16, kind="ExternalInput")
    mw2 = nc.dram_tensor("mw2c", [64, 1], BF16, kind="ExternalInput")
    mb2r = nc.dram_tensor("mb2r", [1, 1], BF16, kind="ExternalInput")
    cu = nc.dram_tensor("cu", [P, CSLOT // 16], I16, kind="ExternalInput")
    cv = nc.dram_tensor("cv", [P, CSLOT // 16], I16, kind="ExternalInput")
    featr = nc.dram_tensor("featr", [1, CSLOT], BF16, kind="ExternalInput")
    maskr = nc.dram_tensor("maskr", [1, CSLOT], BF16, kind="ExternalInput")

    y_out = nc.dram_tensor("y_out", [P, YCOLS], F32, kind="ExternalOutput")
    p_out = nc.dram_tensor("p_out", [P, YCOLS], F32, kind="ExternalOutput")

    hsh = nc.dram_tensor("hsh", [NSH, D], BF16, kind="Internal")
    hfull = nc.dram_tensor("hfull", [NTOT, D], BF16, kind="Internal",
                           addr_space="Shared")
    gqsh = nc.dram_tensor("gqsh", [NSH, D], BF16, kind="Internal")
    gqfull = nc.dram_tensor("gqfull", [NTOT, D], BF16, kind="Internal",
                            addr_space="Shared")
    ysh = nc.dram_tensor("ysh", [1, CSLOT], F32, kind="Internal")
    yfull = nc.dram_tensor("yfull", [NCORE, CSLOT], F32, kind="Internal",
                           addr_space="Shared")

    rg = [list(range(NCORE))]
    nch_u, win_ch0, calls = em["nch_u"], em["win_ch0"], em["calls"]

    with tile.TileContext(nc) as tc:
        with (
            tc.tile_pool(name="const", bufs=1) as cp,
            tc.tile_pool(name="big", bufs=1) as bp,
            tc.tile_pool(name="msgs", bufs=8) as mp,
            tc.tile_pool(name="ohp", bufs=4) as op_,
            tc.tile_pool(name="wrk", bufs=4) as wp,
            tc.tile_pool(name="frows", bufs=3) as fp_,
            tc.tile_pool(name="ps_w", bufs=2, space="PSUM") as ps_w,
            tc.tile_pool(name="ps_l", bufs=2, space="PSUM") as ps_l,
            tc.tile_pool(name="ps_t", bufs=1, space="PSUM") as ps_t,
            tc.tile_pool(name="ps_c", bufs=2, space="PSUM") as ps_c,
            tc.tile_pool(name="ps_y", bufs=1, space="PSUM") as ps_y,
        ):
            nc.gpsimd.load_library(library_config.mlp)

            def load(pool, t, shape=None):
                tl = pool.tile(shape or list(t.shape), t.dtype, tag=t.name)
                nc.sync.dma_start(tl[:], t[:])
                return tl

            gidx_t = load(cp, gidx)
            invd_t = load(cp, invd)
            identb_t = load(cp, identb)
            onesr_t = load(cp, onesr)
            onesc_t = load(cp, onesc)
            e1_t = load(cp, e1)
            e2_t = load(cp, e2)
            wself_t = [load(cp, t) for t in wself]
            wneigh_t = [load(cp, t) for t in wneigh]
            crow_t = [load(cp, t) for t in crow]
            amat_t = load(cp, amat)
            bmat_t = load(cp, bmat)
            mw0r_t = load(cp, mw0r)
            mb0r_t = load(cp, mb0r)
            mw1_t = load(cp, mw1)
            mb1r_t = load(cp, mb1r)
            mw2_t = load(cp, mw2)
            mb2r_t = load(cp, mb2r)
            cu_t = load(cp, cu)
            cv_t = load(cp, cv)

            curT = bp.tile([P, NSH], BF16, tag="curT")
            nxtT = bp.tile([P, NSH], BF16, tag="nxtT")
            aggrb = bp.tile([P, NSH], BF16, tag="aggrb")
            nc.sync.dma_start(curT[:], xT[:])

            qrr = [0]

            def next_q():
                q = qrr[0]
                qrr[0] = (q + 1) % 4
                return q

            for layer in range(2):
                gtab = xb if layer == 0 else hfull
                # window state: psum tile + chunks left
                win_ps = {}
                ci = 0  # call index cursor (calls are in window order)
                for w in range(NWIN):
                    ww = _win_width(w)
                    c0 = int(win_ch0[w])
                    mtot = int(nch_u[w, 0] + nch_u[w, 1])
                    if mtot == 0:
                        nc.vector.memset(aggrb[:, w * W:w * W + ww], 0.0)
                        continue
                    ps = ps_w.tile([P, W], F32, tag="psw")
                    done = 0
                    for s in (0, 1):
                        nch = int(nch_u[w, s])
                        left = nch
                        while left > 0:
                            (p0, n_idx, sh, ch0) = calls[ci]
                            ci += 1
                            ncall = n_idx // P
                            g = mp.tile([P, GCALL // P, P], BF16, tag="g")
                            nc.gpsimd.dma_gather(
                                g[:, :ncall, :],
                                gtab[sh * HALF:(sh + 1) * HALF, :],
                                gidx_t[:, p0 // 16:(p0 + n_idx) // 16],
                                n_idx, n_idx, P, single_packet=False,
                                queue_num=next_q())
                            oht = op_.tile([P, GCALL // P, W], F8, tag="oh")
                            nc.sync.dma_start(
                                oht[:, :ncall, :],
                                ohm[:, ch0 * W:(ch0 + ncall) * W]
                                .rearrange("p (a b) -> p a b", b=W))
                            for cc in range(ncall):
                                nc.tensor.matmul(
                                    ps[:], lhsT=g[:, cc, :], rhs=oht[:, cc, :],
                                    start=(done == 0), stop=(done == mtot - 1))
                                done += 1
                            left -= ncall
                    # fused PSUM->SBUF copy with 1/deg scale
                    nc.vector.tensor_tensor(
                        aggrb[:, w * W:w * W + ww], ps[:, :ww],
                        invd_t[:, w * W:w * W + ww], ALU.mult)
                    # linear phase for completed 128-node blocks
                    for b in range((w * W) // P, (w * W + ww) // P):
                        ph = ps_l.tile([P, P], F32, tag="ph")
                        nc.tensor.matmul(ph[:], lhsT=aggrb[:, b * P:(b + 1) * P],
                                         rhs=wneigh_t[layer][:],
                                         start=True, stop=False)
                        nc.tensor.matmul(ph[:], lhsT=curT[:, b * P:(b + 1) * P],
                                         rhs=wself_t[layer][:],
                                         start=False, stop=False)
                        nc.tensor.matmul(ph[:], lhsT=onesr_t[:],
                                         rhs=crow_t[layer][:],
                                         start=False, stop=True)
                        tmp = wp.tile([P, P], BF16, tag="tmp")
                        nc.vector.tensor_scalar(tmp[:], ph[:], SLOPE, None,
                                                ALU.mult)
                        ht = wp.tile([P, P], BF16, tag="ht")
                        nc.vector.tensor_tensor(ht[:], ph[:], tmp[:], ALU.max)
                        if layer == 0:
                            nc.sync.dma_start(hsh[b * P:(b + 1) * P, :], ht[:])
                        pt = ps_t.tile([P, P], BF16, tag="pt")
                        nc.tensor.transpose(pt[:], ht[:], identb_t[:])
                        nc.scalar.activation(nxtT[:, b * P:(b + 1) * P], pt[:],
                                             AF.Copy)
                if layer == 0:
                    nc.gpsimd.collective_compute(
                        "AllGather", ALU.bypass, replica_groups=rg,
                        ins=[hsh[:].opt()], outs=[hfull[:].opt()])
                curT, nxtT = nxtT, curT

            # ---- g/q tables (cols 0:64 g = h2@A, 64:128 q = h2@B + mb0) ----
            for b in range(NBLK):
                pg = ps_l.tile([P, P], F32, tag="ph")
                nc.tensor.matmul(pg[:, 0:64], lhsT=curT[:, b * P:(b + 1) * P],
                                 rhs=amat_t[:], start=True, stop=True)
                nc.tensor.matmul(pg[:, 64:128], lhsT=curT[:, b * P:(b + 1) * P],
                                 rhs=bmat_t[:], start=True, stop=False)
                nc.tensor.matmul(pg[:, 64:128], lhsT=onesr_t[:], rhs=mb0r_t[:],
                                 start=False, stop=True)
                gq = wp.tile([P, P], BF16, tag="tmp")
                nc.scalar.activation(gq[:], pg[:], AF.Copy)
                nc.sync.dma_start(gqsh[b * P:(b + 1) * P, :], gq[:])
            nc.gpsimd.collective_compute(
                "AllGather", ALU.bypass, replica_groups=rg,
                ins=[gqsh[:].opt()], outs=[gqfull[:].opt()])

            # ---- transposed candidate gathers ----
            guT = bp.tile([P, CSLOT], BF16, tag="guT")
            gvT = bp.tile([P, CSLOT], BF16, tag="gvT")
            tail0 = max(e + n for (e, n, _s) in cm["ucalls"])
            if tail0 < CSLOT:
                nc.vector.memset(guT[:, tail0:], 0.0)
                nc.vector.memset(gvT[:, tail0:], 0.0)
            for (tl, idx_t, cl) in ((guT, cu_t, cm["ucalls"]),
                                    (gvT, cv_t, cm["vcalls"])):
                for (s0, n_idx, sh) in cl:
                    nc.gpsimd.dma_gather(
                        tl[:, s0:s0 + n_idx].rearrange("p (a b) -> p a b", a=1),
                        gqfull[sh * HALF:(sh + 1) * HALF, :],
                        idx_t[:, s0 // 16:(s0 + n_idx) // 16],
                        n_idx, n_idx, P, transpose=True, single_packet=False,
                        queue_num=0)
            # ---- candidate MLP (feat on partitions, 512-wide chunks) ----
            for c in range(NCC):
                sl = slice(c * CCH, (c + 1) * CCH)
                fr = fp_.tile([1, CCH], BF16, tag="fr")
                nc.sync.dma_start(fr[:], featr[:, sl])
                mr = fp_.tile([1, CCH], BF16, tag="mr")
                nc.sync.dma_start(mr[:], maskr[:, sl])
                psf = ps_c.tile([64, CCH], F32, tag="psc")
                nc.tensor.matmul(psf[:], lhsT=e1_t[:], rhs=guT[:, sl],
                                 start=True, stop=False)
                nc.tensor.matmul(psf[:], lhsT=e2_t[:], rhs=gvT[:, sl],
                                 start=False, stop=False)
                nc.tensor.matmul(psf[:], lhsT=mw0r_t[:], rhs=fr[:],
                                 start=False, stop=True)
                zs = wp.tile([64, CCH], BF16, tag="zs")
                nc.vector.tensor_scalar(zs[:], psf[:], SLOPE, None, ALU.mult)
                z1 = wp.tile([64, CCH], BF16, tag="z1")
                nc.vector.tensor_tensor(z1[:], psf[:], zs[:], ALU.max)
                psz = ps_c.tile([64, CCH], F32, tag="psc")
                nc.tensor.matmul(psz[:], lhsT=mw1_t[:], rhs=z1[:],
                                 start=True, stop=False)
                nc.tensor.matmul(psz[:], lhsT=mb1r_t[:], rhs=onesc_t[:],
                                 start=False, stop=True)
                zs2 = wp.tile([64, CCH], BF16, tag="zs")
                nc.vector.tensor_scalar(zs2[:], psz[:], SLOPE, None, ALU.mult)
                z2 = wp.tile([64, CCH], BF16, tag="z1")
                nc.vector.tensor_tensor(z2[:], psz[:], zs2[:], ALU.max)
                py = ps_y.tile([1, CCH], F32, tag="psy")
                nc.tensor.matmul(py[:], lhsT=mw2_t[:], rhs=z2[:],
                                 start=True, stop=False)
                nc.tensor.matmul(py[:], lhsT=mb2r_t[:], rhs=onesc_t[:],
                                 start=False, stop=True)
                ym = fp_.tile([1, CCH], F32, tag="ym")
                nc.vector.tensor_tensor(ym[:], py[:], mr[:], ALU.add)
                nc.sync.dma_start(ysh[:, sl], ym[:])
            nc.gpsimd.collective_compute(
                "AllGather", ALU.bypass, replica_groups=rg,
                ins=[ysh[:].opt()], outs=[yfull[:].opt()])
            # ---- softmax over yfull viewed as [128, YCOLS] ----
            yf = bp.tile([P, YCOLS], F32, tag="yf")
            nc.sync.dma_start(yf[:], yfull[:].rearrange("a b -> (a b)")
                              .rearrange("(p c) -> p c", p=P))
            nc.sync.dma_start(y_out[:], yf[:])
            rmax = wp.tile([P, 1], F32, tag="rmax")
            nc.vector.tensor_reduce(rmax[:], yf[:], mybir.AxisListType.X, ALU.max)
            gmax = wp.tile([P, 1], F32, tag="gmax")
            nc.gpsimd.partition_all_reduce(gmax[:], rmax[:], P,
                                           bass_isa.ReduceOp.max)
            ngmax = wp.tile([P, 1], F32, tag="ngmax")
            nc.vector.tensor_scalar(ngmax[:], gmax[:], -1.0, None, ALU.mult)
            ef = bp.tile([P, YCOLS], F32, tag="ef")
            se = wp.tile([P, 1], F32, tag="se")
            nc.scalar.activation(ef[:], yf[:], AF.Exp, bias=ngmax[:, 0:1],
                                 accum_out=se[:])
            stot = wp.tile([P, 1], F32, tag="stot")
            nc.gpsimd.partition_all_reduce(stot[:], se[:], P,
                                           bass_isa.ReduceOp.add)
            invs = wp.tile([P, 1], F32, tag="invs")
            nc.vector.reciprocal(invs[:], stot[:])
            pf = bp.tile([P, YCOLS], F32, tag="pf")
            nc.vector.tensor_scalar(pf[:], ef[:], invs[:, 0:1], None, ALU.mult)
            nc.sync.dma_start(p_out[:], pf[:])
    nc.compile()
    return nc


def kernel(x, src, dst, cand_u, cand_v, cand_feat,
           w_self0, w_neigh0, b0, gamma0, beta0, rm0, rv0,
           w_self1, w_neigh1, b1, gamma1, beta1, rm1, rv1,
           mw0, mb0, mw1, mb1, mw2, mb2):
    x = np.asarray(x, np.float32)
    src = np.asarray(src, np.int64)
    dst = np.asarray(dst, np.int64)
    cand_u = np.asarray(cand_u, np.int64)
    cand_v = np.asarray(cand_v, np.int64)
    cand_feat = np.asarray(cand_feat, np.float32)

    deg = np.bincount(dst, minlength=N).astype(np.float32)
    invdeg = 1.0 / np.maximum(deg, 1.0)
    em, edata = _prep_edges(src, dst)
    cm, cdata = _prep_cands(cand_u, cand_v, cand_feat)

    xpad = np.zeros((NTOT, D), np.float32)
    xpad[:N] = x
    invdp = np.zeros(NTOT, np.float32)
    invdp[:N] = invdeg

    com = {
        "xb": xpad.astype(BF),
        "identb": np.eye(P, dtype=np.float32).astype(BF),
        "onesr": np.ones((1, P), BF),
        "onesc": np.ones((1, CCH), BF),
        "e1": np.concatenate([np.eye(64), np.zeros((64, 64))], 0).astype(BF),
        "e2": np.concatenate([np.zeros((64, 64)), np.eye(64)], 0).astype(BF),
    }
    for l, (ws, wn, b, ga, be, rme, rve) in enumerate(
        ((w_self0, w_neigh0, b0, gamma0, beta0, rm0, rv0),
         (w_self1, w_neigh1, b1, gamma1, beta1, rm1, rv1))):
        a = (np.asarray(ga) / np.sqrt(np.asarray(rve) + BN_EPS)).astype(np.float32)
        com[f"wself{l}"] = (np.asarray(ws) * a[None, :]).astype(BF)
        com[f"wneigh{l}"] = (np.asarray(wn) * a[None, :]).astype(BF)
        com[f"crow{l}"] = (a * (np.asarray(b) - np.asarray(rme)) + np.asarray(be)).astype(BF)[None, :]
    com["amat"] = np.asarray(mw0[0:128], np.float32).astype(BF)
    com["bmat"] = np.asarray(mw0[128:256], np.float32).astype(BF)
    com["mw0r"] = np.asarray(mw0[256], np.float32).astype(BF)[None, :]
    com["mb0r"] = np.asarray(mb0, np.float32).astype(BF)[None, :]
    com["mw1"] = np.asarray(mw1, np.float32).astype(BF)
    com["mb1r"] = np.asarray(mb1, np.float32).astype(BF)[None, :]
    com["mw2c"] = np.asarray(mw2, np.float32).astype(BF)
    com["mb2r"] = np.asarray(mb2, np.float32).reshape(1, 1).astype(BF)

    nc = _build_nc(em, cm)
    in_maps = []
    for k in range(NCORE):
        m = dict(com)
        m["xT"] = xpad[k * NSH:(k + 1) * NSH].T.astype(BF).copy()
        m["invd"] = np.tile(invdp[k * NSH:(k + 1) * NSH].astype(BF), (P, 1))
        m["gidx"] = edata[k]["gidx"]
        m["ohm"] = edata[k]["ohm"]
        m["cu"] = cdata[k]["cu"]
        m["cv"] = cdata[k]["cv"]
        m["featr"] = cdata[k]["feat"]
        m["maskr"] = cdata[k]["mask"]
        in_maps.append(m)
    trace = bool(os.environ.get("KERNEL_TRACE"))
    if trace:
        import types
        if "antenv.axon_hooks" not in sys.modules:
            try:
                import antenv
                from trn_agent_boot.trn_boot import _ntff_profile_via_ctypes
                mod = types.ModuleType("antenv.axon_hooks")
                hook = [_ntff_profile_via_ctypes("/opt/axon/libaxon_pjrt.so")]
                mod.set_axon_ntff_profile_hook = lambda h: hook.__setitem__(0, h)
                mod.get_axon_ntff_profile_hook = lambda: hook[0]
                sys.modules["antenv.axon_hooks"] = mod
                antenv.axon_hooks = mod
            except Exception:
                trace = False
    res = run_bass_kernel_spmd(nc, in_maps, core_ids=list(range(NCORE)),
                               trace=trace,
                               tmpdir=os.environ.get("KERNEL_TRACE_DIR"))
    if trace and res.exec_time_ns is not None:
        print(f"HW exec time: {res.exec_time_ns} ns")
    y_all = np.zeros(C, np.float32)
    p_all = np.zeros(C, np.float32)
    cslot = cm["cslot"]
    y_lin = res.results[0]["y_out"].ravel()   # flat order: core, slot
    p_lin = res.results[0]["p_out"].ravel()
    for k in range(NCORE):
        sm = cdata[k]["slotmap"]
        valid = sm >= 0
        j = np.nonzero(valid)[0]
        y_all[sm[valid]] = y_lin[k * cslot + j]
        p_all[sm[valid]] = p_lin[k * cslot + j]
    return y_all[:, None], p_all[:, None]


# revision 9
# speedup vs baseline: 2.6710x; 1.0795x over previous
"""Trainium2 Bass kernel for nn_PolicyNetwork3 (2-layer GraphSAGE + edge-MLP).

v2 design (8 NeuronCores, SPMD single NEFF):
- dst-sharded aggregation; core k owns node block [6272k, 6272k+6272).
- Edge messages gathered per-edge from bf16 HBM row tables via 4-queue
  round-robin dma_gather (descriptor generation parallelizes across the
  SWDGE queues; ~2.4ns/row vs 8ns serialized).
- segment-sum per 256-dst window via one-hot matmuls (bf16 gathered rows x
  fp8 0/1 one-hot streamed from HBM) accumulating in PSUM across the LO/HI
  half-table runs; per-window PSUM->SBUF copy fuses the 1/deg scale.
- BN folded into SAGE weights on host; linear+leaky per 128-node block;
  h shards exchanged with bf16 AllGather.
- candidate MLP: g/q tables per node; transposed candidate gathers put
  features on partitions so the whole MLP runs as 512-wide PE matmuls
  (identity-matmul adds, K=1 bias rows); global softmax on-device.
"""

import os
import sys

sys.path.insert(0, "/opt/trn_rl_repo")
sys.path.insert(0, "/root/.axon_site")

import numpy as np
import ml_dtypes

import concourse.bacc as bacc
import concourse.bass as bass
import concourse.bass_isa as bass_isa
import concourse.mybir as mybir
import concourse.tile as tile
from concourse import library_config
from concourse.bass_utils import run_bass_kernel_spmd

P = 128
N, E, C = 50000, 800000, 100000
D = 128
NCORE = 8
NSH = 6272            # nodes per core shard
NTOT = NSH * NCORE    # 50176 padded node table
HALF = NTOT // 2      # 25088 rows per gather-table half
W = 256               # dst window width (PSUM cols)
NWIN = (NSH + W - 1) // W   # 25 windows (last one 128 wide)
NBLK = NSH // P       # 49 node blocks for the linear phase
CSH = C // NCORE      # 12500 candidates per core
GCALL = 2048          # max idxs per dma_gather call
CCH = 512             # candidate MLP chunk
BN_EPS = 1e-5
SLOPE = 0.01
F32 = mybir.dt.float32
BF16 = mybir.dt.bfloat16
F8 = mybir.dt.float8e4
I16 = mybir.dt.int16
AF = mybir.ActivationFunctionType
ALU = mybir.AluOpType
BF = ml_dtypes.bfloat16
F8NP = ml_dtypes.float8_e4m3fn


def _wrap16(idx_lin):
    """[n] -> [128, n/16] int16 in the dma_gather wrapped+replicated layout."""
    n = idx_lin.shape[0]
    assert n % 16 == 0
    w = idx_lin.reshape(n // 16, 16).T.astype(np.int16)
    return np.tile(w, (8, 1)).copy()


def _win_width(w):
    return min(W, NSH - w * W)


def _prep_edges(src, dst):
    """Uniform per-core window/run/chunk schedule + per-core idx and one-hot."""
    core = dst // NSH
    winl = (dst - core * NSH) // W
    dstloc = (dst - core * NSH) - winl * W
    half = (src >= HALF).astype(np.int64)
    key = (core * NWIN + winl) * 2 + half
    order = np.argsort(key, kind="stable")
    cnt = np.bincount(key, minlength=NCORE * NWIN * 2).reshape(NCORE, NWIN, 2)
    nch_u = (-(-cnt // P)).max(axis=0)        # [NWIN, 2] uniform chunk counts
    # global chunk order: w0 LO chunks, w0 HI, w1 LO, ...
    win_ch0 = np.zeros(NWIN + 1, np.int64)
    np.cumsum(nch_u.sum(axis=1), out=win_ch0[1:])
    totch = int(win_ch0[-1])
    nslot = totch * P
    # gather calls (uniform): per (w, half) run split into <=GCALL
    calls = []  # (slot_start, n_idx, half, chunk0)
    for w in range(NWIN):
        c0 = int(win_ch0[w])
        for s in (0, 1):
            nch = int(nch_u[w, s])
            p0, left = c0 * P, nch * P
            while left > 0:
                n = min(GCALL, left)
                calls.append((p0, n, s, p0 // P))
                p0 += n
                left -= n
            c0 += nch
    bstart = np.zeros(NCORE * NWIN * 2 + 1, np.int64)
    np.cumsum(np.bincount(key, minlength=NCORE * NWIN * 2), out=bstart[1:])
    gidx = np.zeros((NCORE, nslot), np.int16)
    ohm = np.zeros((NCORE, P, totch * W), np.uint8)  # fp8 bits (1.0 = 0x38)
    ONE = np.float32(1.0).astype(F8NP).view(np.uint8)
    for k in range(NCORE):
        for w in range(NWIN):
            pos = int(win_ch0[w]) * P
            for s in (0, 1):
                b = (k * NWIN + w) * 2 + s
                sl = order[bstart[b]:bstart[b + 1]]
                n = len(sl)
                gidx[k, pos:pos + n] = (src[sl] - s * HALF).astype(np.int16)
                slots = pos + np.arange(n)
                ohm[k, slots % P, (slots // P) * W + dstloc[sl]] = ONE
                pos += int(nch_u[w, s]) * P
    meta = dict(nch_u=nch_u, win_ch0=win_ch0, totch=totch, nslot=nslot,
                calls=calls)
    data = [dict(gidx=_wrap16(gidx[k]), ohm=ohm[k].view(F8NP)) for k in range(NCORE)]
    return meta, data


def _prep_cands(cand_u, cand_v, cand_feat):
    """Shard candidates, group by (u_half, v_half), pad groups to 128."""
    gch = np.zeros((NCORE, 4), np.int64)
    groups = [[None] * 4 for _ in range(NCORE)]
    for k in range(NCORE):
        ids = np.arange(k * CSH, (k + 1) * CSH)
        g = (cand_u[ids] >= HALF) * 2 + (cand_v[ids] >= HALF)
        for gi in range(4):
            groups[k][gi] = ids[g == gi]
            gch[k, gi] = -(-len(groups[k][gi]) // P)
    gch_u = gch.max(axis=0)
    goff = np.zeros(5, np.int64)
    np.cumsum(gch_u * P, out=goff[1:])
    cslot = int(goff[4])                       # 128-granular
    cu = np.zeros((NCORE, cslot), np.int16)
    cv = np.zeros((NCORE, cslot), np.int16)
    ft = np.zeros((NCORE, cslot), BF)
    mask = np.full((NCORE, cslot), -1e30, np.float32).astype(BF)
    slotmap = np.full((NCORE, cslot), -1, np.int64)
    for k in range(NCORE):
        for gi in range(4):
            ids = groups[k][gi]
            n = len(ids)
            p0 = int(goff[gi])
            uh, vh = gi // 2, gi % 2
            cu[k, p0:p0 + n] = (cand_u[ids] - uh * HALF).astype(np.int16)
            cv[k, p0:p0 + n] = (cand_v[ids] - vh * HALF).astype(np.int16)
            ft[k, p0:p0 + n] = cand_feat[ids, 0].astype(BF)
            mask[k, p0:p0 + n] = 0.0
            slotmap[k, p0:p0 + n] = ids
    # u calls: groups 0-1 (uh=0) then 2-3 (uh=1); v calls per group
    ucalls, vcalls = [], []
    for s, lo, hi in ((0, 0, int(goff[2])), (1, int(goff[2]), int(goff[4]))):
        p = lo
        while p < hi:
            n = min(GCALL, hi - p)
            ucalls.append((p, n, s))
            p += n
    for gi in range(4):
        p, hi = int(goff[gi]), int(goff[gi + 1])
        while p < hi:
            n = min(GCALL, hi - p)
            vcalls.append((p, n, gi % 2))
            p += n
    meta = dict(cslot=cslot, ucalls=ucalls, vcalls=vcalls)
    data = [dict(cu=_wrap16(cu[k]), cv=_wrap16(cv[k]), feat=ft[k][None, :],
                 mask=mask[k][None, :], slotmap=slotmap[k]) for k in range(NCORE)]
    return meta, data


def _build_nc(em, cm):
    nc = bacc.Bacc("TRN2", target_bir_lowering=False, debug=False,
                   num_devices=NCORE, num_swdge_queues=4)
    TOTCH, NSLOT = em["totch"], em["nslot"]
    CSLOT = cm["cslot"]
    NCC = -(-CSLOT // CCH)                  # candidate MLP chunk groups
    YCOLS = NCORE * CSLOT // P              # yfull viewed as [128, YCOLS]

    xb = nc.dram_tensor("xb", [NTOT, D], BF16, kind="ExternalInput")
    xT = nc.dram_tensor("xT", [P, NSH], BF16, kind="ExternalInput")
    gidx = nc.dram_tensor("gidx", [P, NSLOT // 16], I16, kind="ExternalInput")
    ohm = nc.dram_tensor("ohm", [P, TOTCH * W], F8, kind="ExternalInput")
    invd = nc.dram_tensor("invd", [P, NSH], BF16, kind="ExternalInput")
    wself = [nc.dram_tensor(f"wself{l}", [D, D], BF16, kind="ExternalInput") for l in range(2)]
    wneigh = [nc.dram_tensor(f"wneigh{l}", [D, D], BF16, kind="ExternalInput") for l in range(2)]
    crow = [nc.dram_tensor(f"crow{l}", [1, D], BF16, kind="ExternalInput") for l in range(2)]
    identb = nc.dram_tensor("identb", [P, P], BF16, kind="ExternalInput")
    onesr = nc.dram_tensor("onesr", [1, P], BF16, kind="ExternalInput")
    onesc = nc.dram_tensor("onesc", [1, CCH], BF16, kind="ExternalInput")
    amat = nc.dram_tensor("amat", [D, 64], BF16, kind="ExternalInput")
    bmat = nc.dram_tensor("bmat", [D, 64], BF16, kind="ExternalInput")
    mw0r = nc.dram_tensor("mw0r", [1, 64], BF16, kind="ExternalInput")
    mb0r = nc.dram_tensor("mb0r", [1, 64], BF16, kind="ExternalInput")
    mw1 = nc.dram_tensor("mw1", [64, 64], BF16, kind="ExternalInput")
    mb1r = nc.dram_tensor("mb1r", [1, 64], BF16, kind="ExternalInput")
    mw2 = nc.dram_tensor("mw2c", [64, 1], BF16, kind="ExternalInput")
    mb2r = nc.dram_tensor("mb2r", [1, 1], BF16, kind="ExternalInput")
    cu = nc.dram_tensor("cu", [P, CSLOT // 16], I16, kind="ExternalInput")
    cv = nc.dram_tensor("cv", [P, CSLOT // 16], I16, kind="ExternalInput")
    featr = nc.dram_tensor("featr", [1, CSLOT], BF16, kind="ExternalInput")
    maskr = nc.dram_tensor("maskr", [1, CSLOT], BF16, kind="ExternalInput")

    y_out = nc.dram_tensor("y_out", [P, YCOLS], F32, kind="ExternalOutput")
    p_out = nc.dram_tensor("p_out", [P, YCOLS], F32, kind="ExternalOutput")

    hsh = nc.dram_tensor("hsh", [NSH, D], BF16, kind="Internal")
    hfull = nc.dram_tensor("hfull", [NTOT, D], BF16, kind="Internal",
                           addr_space="Shared")
    gqsh = nc.dram_tensor("gqsh", [NSH, D], BF16, kind="Internal")
    gqfull = nc.dram_tensor("gqfull", [NTOT, D], BF16, kind="Internal",
                            addr_space="Shared")
    ysh = nc.dram_tensor("ysh", [1, CSLOT], F32, kind="Internal")
    yfull = nc.dram_tensor("yfull", [NCORE, CSLOT], F32, kind="Internal",
                           addr_space="Shared")

    rg = [list(range(NCORE))]
    nch_u, win_ch0, calls = em["nch_u"], em["win_ch0"], em["calls"]

    with tile.TileContext(nc) as tc:
        with (
            tc.tile_pool(name="const", bufs=1) as cp,
            tc.tile_pool(name="big", bufs=1) as bp,
            tc.tile_pool(name="msgs", bufs=10) as mp,
            tc.tile_pool(name="ohp", bufs=4) as op_,
            tc.tile_pool(name="wrk", bufs=4) as wp,
            tc.tile_pool(name="frows", bufs=3) as fp_,
            tc.tile_pool(name="gup", bufs=3) as gup,
            tc.tile_pool(name="gvp", bufs=3) as gvp,
            tc.tile_pool(name="z1p", bufs=2) as z1p,
        ):
            nc.gpsimd.load_library(library_config.mlp)

            def load(pool, t, shape=None):
                tl = pool.tile(shape or list(t.shape), t.dtype, tag=t.name)
                nc.sync.dma_start(tl[:], t[:])
                return tl

            gidx_t = load(cp, gidx)
            invd_t = load(cp, invd)
            identb_t = load(cp, identb)
            onesr_t = load(cp, onesr)
            onesc_t = load(cp, onesc)
            wself_t = [load(cp, t) for t in wself]
            wneigh_t = [load(cp, t) for t in wneigh]
            crow_t = [load(cp, t) for t in crow]
            amat_t = load(cp, amat)
            bmat_t = load(cp, bmat)
            mw0r_t = load(cp, mw0r)
            mb0r_t = load(cp, mb0r)
            mw1_t = load(cp, mw1)
            mb1r_t = load(cp, mb1r)
            mw2_t = load(cp, mw2)
            mb2r_t = load(cp, mb2r)
            cu_t = load(cp, cu)
            cv_t = load(cp, cv)
            featr_t = load(cp, featr)

            curT = bp.tile([P, NSH], BF16, tag="curT")
            nxtT = bp.tile([P, NSH], BF16, tag="nxtT")
            aggrb = bp.tile([P, NSH], BF16, tag="aggrb")
            nc.sync.dma_start(curT[:], xT[:])

            qrr = [0]

            def next_q():
                q = qrr[0]
                qrr[0] = (q + 1) % 4
                return q

            ctxA = tc.tile_pool(name="ps_w", bufs=2, space="PSUM")
            ps_w = ctxA.__enter__()
            ctxB = tc.tile_pool(name="ps_l", bufs=2, space="PSUM")
            ps_l = ctxB.__enter__()
            ctxC = tc.tile_pool(name="ps_t", bufs=1, space="PSUM")
            ps_t = ctxC.__enter__()

            for layer in range(2):
                gtab = xb if layer == 0 else hfull
                # window state: psum tile + chunks left
                win_ps = {}
                ci = 0  # call index cursor (calls are in window order)
                for w in range(NWIN):
                    ww = _win_width(w)
                    c0 = int(win_ch0[w])
                    mtot = int(nch_u[w, 0] + nch_u[w, 1])
                    if mtot == 0:
                        nc.vector.memset(aggrb[:, w * W:w * W + ww], 0.0)
                        continue
                    ps = ps_w.tile([P, W], F32, tag="psw")
                    done = 0
                    for s in (0, 1):
                        nch = int(nch_u[w, s])
                        left = nch
                        while left > 0:
                            (p0, n_idx, sh, ch0) = calls[ci]
                            ci += 1
                            ncall = n_idx // P
                            g = mp.tile([P, GCALL // P, P], BF16, tag="g")
                            nc.gpsimd.dma_gather(
                                g[:, :ncall, :],
                                gtab[sh * HALF:(sh + 1) * HALF, :],
                                gidx_t[:, p0 // 16:(p0 + n_idx) // 16],
                                n_idx, n_idx, P, single_packet=False,
                                queue_num=next_q())
                            oht = op_.tile([P, GCALL // P, W], F8, tag="oh")
                            nc.sync.dma_start(
                                oht[:, :ncall, :],
                                ohm[:, ch0 * W:(ch0 + ncall) * W]
                                .rearrange("p (a b) -> p a b", b=W))
                            for cc in range(ncall):
                                nc.tensor.matmul(
                                    ps[:], lhsT=g[:, cc, :], rhs=oht[:, cc, :],
                                    start=(done == 0), stop=(done == mtot - 1))
                                done += 1
                            left -= ncall
                    # fused PSUM->SBUF copy with 1/deg scale
                    nc.vector.tensor_tensor(
                        aggrb[:, w * W:w * W + ww], ps[:, :ww],
                        invd_t[:, w * W:w * W + ww], ALU.mult)
                    # linear phase for completed 128-node blocks
                    for b in range((w * W) // P, (w * W + ww) // P):
                        ph = ps_l.tile([P, P], F32, tag="ph")
                        nc.tensor.matmul(ph[:], lhsT=aggrb[:, b * P:(b + 1) * P],
                                         rhs=wneigh_t[layer][:],
                                         start=True, stop=False)
                        nc.tensor.matmul(ph[:], lhsT=curT[:, b * P:(b + 1) * P],
                                         rhs=wself_t[layer][:],
                                         start=False, stop=False)
                        nc.tensor.matmul(ph[:], lhsT=onesr_t[:],
                                         rhs=crow_t[layer][:],
                                         start=False, stop=True)
                        tmp = wp.tile([P, P], BF16, tag="tmp")
                        nc.vector.tensor_scalar(tmp[:], ph[:], SLOPE, None,
                                                ALU.mult)
                        ht = wp.tile([P, P], BF16, tag="ht")
                        nc.vector.tensor_tensor(ht[:], ph[:], tmp[:], ALU.max)
                        if layer == 0:
                            nc.sync.dma_start(hsh[b * P:(b + 1) * P, :], ht[:])
                        pt = ps_t.tile([P, P], BF16, tag="pt")
                        nc.tensor.transpose(pt[:], ht[:], identb_t[:])
                        nc.scalar.activation(nxtT[:, b * P:(b + 1) * P], pt[:],
                                             AF.Copy)
                if layer == 0:
                    nc.gpsimd.collective_compute(
                        "AllGather", ALU.bypass, replica_groups=rg,
                        ins=[hsh[:].opt()], outs=[hfull[:].opt()])
                curT, nxtT = nxtT, curT

            # ---- g/q tables (cols 0:64 g = h2@A, 64:128 q = h2@B + mb0) ----
            for b in range(NBLK):
                pg = ps_l.tile([P, P], F32, tag="ph")
                nc.tensor.matmul(pg[:, 0:64], lhsT=curT[:, b * P:(b + 1) * P],
                                 rhs=amat_t[:], start=True, stop=True)
                nc.tensor.matmul(pg[:, 64:128], lhsT=curT[:, b * P:(b + 1) * P],
                                 rhs=bmat_t[:], start=True, stop=False)
                nc.tensor.matmul(pg[:, 64:128], lhsT=onesr_t[:], rhs=mb0r_t[:],
                                 start=False, stop=True)
                gq = wp.tile([P, P], BF16, tag="tmp")
                nc.scalar.activation(gq[:], pg[:], AF.Copy)
                nc.sync.dma_start(gqsh[b * P:(b + 1) * P, :], gq[:])
            nc.gpsimd.collective_compute(
                "AllGather", ALU.bypass, replica_groups=rg,
                ins=[gqsh[:].opt()], outs=[gqfull[:].opt()])

            ctxC.__exit__(None, None, None)
            ctxB.__exit__(None, None, None)
            ctxA.__exit__(None, None, None)
            with (
                tc.tile_pool(name="psc1", bufs=2, space="PSUM") as psc1,
                tc.tile_pool(name="psc2", bufs=2, space="PSUM") as psc2,
                tc.tile_pool(name="ps_c", bufs=2, space="PSUM") as ps_c,
                tc.tile_pool(name="ps_y", bufs=1, space="PSUM") as ps_y,
            ):
                # ---- candidate gathers (slot layout, 4-queue) ----
                ucalls, vcalls = cm["ucalls"], cm["vcalls"]
                uci, vci = [], []
                for i, (s0, n, sh) in enumerate(ucalls):
                    uci += [(i, j) for j in range(n // P)]
                for i, (s0, n, sh) in enumerate(vcalls):
                    vci += [(i, j) for j in range(n // P)]
                gu_tiles = [None] * len(ucalls)
                gv_tiles = [None] * len(vcalls)
                for i in range(max(len(ucalls), len(vcalls))):
                    for (tiles, calls, pool, idx_t) in (
                            (gu_tiles, ucalls, gup, cu_t),
                            (gv_tiles, vcalls, gvp, cv_t)):
                        if i >= len(calls):
                            continue
                        (s0, n_idx, sh) = calls[i]
                        ncall = n_idx // P
                        g = pool.tile([P, GCALL // P, P], BF16, tag="g")
                        nc.gpsimd.dma_gather(
                            g[:, :ncall, :],
                            gqfull[sh * HALF:(sh + 1) * HALF, :],
                            idx_t[:, s0 // 16:(s0 + n_idx) // 16],
                            n_idx, n_idx, P, single_packet=False,
                            queue_num=next_q())
                        tiles[i] = g
                # ---- candidate MLP ----
                nchunk = CSLOT // P
                z1t = None
                for c in range(nchunk):
                    ui, uo = uci[c]
                    vi, vo = vci[c]
                    psf = psc1.tile([P, 64], F32, tag="psf")
                    nc.tensor.matmul(psf[:], lhsT=identb_t[:],
                                     rhs=gu_tiles[ui][:, uo, 0:64],
                                     start=True, stop=False)
                    nc.tensor.matmul(psf[:], lhsT=identb_t[:],
                                     rhs=gv_tiles[vi][:, vo, 64:128],
                                     start=False, stop=False)
                    nc.tensor.matmul(psf[:], lhsT=featr_t[:, c * P:(c + 1) * P],
                                     rhs=mw0r_t[:], start=False, stop=True)
                    zs = wp.tile([P, 64], BF16, tag="zs")
                    nc.vector.tensor_scalar(zs[:], psf[:], SLOPE, None, ALU.mult)
                    z1b = wp.tile([P, 64], BF16, tag="z1b")
                    nc.vector.tensor_tensor(z1b[:], psf[:], zs[:], ALU.max)
                    ptz = psc2.tile([64, P], BF16, tag="ptz")
                    nc.tensor.transpose(ptz[:], z1b[:], identb_t[:])
                    if c % 4 == 0:
                        z1t = z1p.tile([64, CCH], BF16, tag="z1t")
                    nc.scalar.activation(z1t[:, (c % 4) * P:(c % 4 + 1) * P],
                                         ptz[:], AF.Copy)
                    if c % 4 == 3 or c == nchunk - 1:
                        g0 = (c // 4) * CCH
                        gw = min(CCH, CSLOT - g0)
                        mr = fp_.tile([1, CCH], BF16, tag="mr")
                        nc.sync.dma_start(mr[:, :gw], maskr[:, g0:g0 + gw])
                        psz = ps_c.tile([64, CCH], F32, tag="psc")
                        nc.tensor.matmul(psz[:, :gw], lhsT=mw1_t[:],
                                         rhs=z1t[:, :gw], start=True, stop=False)
                        nc.tensor.matmul(psz[:, :gw], lhsT=mb1r_t[:],
                                         rhs=onesc_t[:, :gw],
                                         start=False, stop=True)
                        zs2 = wp.tile([64, CCH], BF16, tag="zs2")
                        nc.vector.tensor_scalar(zs2[:, :gw], psz[:, :gw],
                                                SLOPE, None, ALU.mult)
                        z2 = wp.tile([64, CCH], BF16, tag="z2")
                        nc.vector.tensor_tensor(z2[:, :gw], psz[:, :gw],
                                                zs2[:, :gw], ALU.max)
                        py = ps_y.tile([1, CCH], F32, tag="psy")
                        nc.tensor.matmul(py[:, :gw], lhsT=mw2_t[:],
                                         rhs=z2[:, :gw], start=True, stop=False)
                        nc.tensor.matmul(py[:, :gw], lhsT=mb2r_t[:],
                                         rhs=onesc_t[:, :gw],
                                         start=False, stop=True)
                        ym = fp_.tile([1, CCH], F32, tag="ym")
                        nc.vector.tensor_tensor(ym[:, :gw], py[:, :gw],
                                                mr[:, :gw], ALU.add)
                        nc.sync.dma_start(ysh[:, g0:g0 + gw], ym[:, :gw])
            nc.gpsimd.collective_compute(
                "AllGather", ALU.bypass, replica_groups=rg,
                ins=[ysh[:].opt()], outs=[yfull[:].opt()])
            # ---- softmax over yfull viewed as [128, YCOLS] ----
            yf = bp.tile([P, YCOLS], F32, tag="yf")
            nc.sync.dma_start(yf[:], yfull[:].rearrange("a b -> (a b)")
                              .rearrange("(p c) -> p c", p=P))
            nc.sync.dma_start(y_out[:], yf[:])
            rmax = wp.tile([P, 1], F32, tag="rmax")
            nc.vector.tensor_reduce(rmax[:], yf[:], mybir.AxisListType.X, ALU.max)
            gmax = wp.tile([P, 1], F32, tag="gmax")
            nc.gpsimd.partition_all_reduce(gmax[:], rmax[:], P,
                                           bass_isa.ReduceOp.max)
            ngmax = wp.tile([P, 1], F32, tag="ngmax")
            nc.vector.tensor_scalar(ngmax[:], gmax[:], -1.0, None, ALU.mult)
            ef = bp.tile([P, YCOLS], F32, tag="ef")
            se = wp.tile([P, 1], F32, tag="se")
            nc.scalar.activation(ef[:], yf[:], AF.Exp, bias=ngmax[:, 0:1],
                                 accum_out=se[:])
            stot = wp.tile([P, 1], F32, tag="stot")
            nc.gpsimd.partition_all_reduce(stot[:], se[:], P,
                                           bass_isa.ReduceOp.add)
            invs = wp.tile([P, 1], F32, tag="invs")
            nc.vector.reciprocal(invs[:], stot[:])
            pf = bp.tile([P, YCOLS], F32, tag="pf")
            nc.vector.tensor_scalar(pf[:], ef[:], invs[:, 0:1], None, ALU.mult)
            nc.sync.dma_start(p_out[:], pf[:])
    nc.compile()
    return nc


def kernel(x, src, dst, cand_u, cand_v, cand_feat,
           w_self0, w_neigh0, b0, gamma0, beta0, rm0, rv0,
           w_self1, w_neigh1, b1, gamma1, beta1, rm1, rv1,
           mw0, mb0, mw1, mb1, mw2, mb2):
    x = np.asarray(x, np.float32)
    src = np.asarray(src, np.int64)
    dst = np.asarray(dst, np.int64)
    cand_u = np.asarray(cand_u, np.int64)
    cand_v = np.asarray(cand_v, np.int64)
    cand_feat = np.asarray(cand_feat, np.float32)

    deg = np.bincount(dst, minlength=N).astype(np.float32)
    invdeg = 1.0 / np.maximum(deg, 1.0)
    em, edata = _prep_edges(src, dst)
    cm, cdata = _prep_cands(cand_u, cand_v, cand_feat)

    xpad = np.zeros((NTOT, D), np.float32)
    xpad[:N] = x
    invdp = np.zeros(NTOT, np.float32)
    invdp[:N] = invdeg

    com = {
        "xb": xpad.astype(BF),
        "identb": np.eye(P, dtype=np.float32).astype(BF),
        "onesr": np.ones((1, P), BF),
        "onesc": np.ones((1, CCH), BF),
    }
    for l, (ws, wn, b, ga, be, rme, rve) in enumerate(
        ((w_self0, w_neigh0, b0, gamma0, beta0, rm0, rv0),
         (w_self1, w_neigh1, b1, gamma1, beta1, rm1, rv1))):
        a = (np.asarray(ga) / np.sqrt(np.asarray(rve) + BN_EPS)).astype(np.float32)
        com[f"wself{l}"] = (np.asarray(ws) * a[None, :]).astype(BF)
        com[f"wneigh{l}"] = (np.asarray(wn) * a[None, :]).astype(BF)
        com[f"crow{l}"] = (a * (np.asarray(b) - np.asarray(rme)) + np.asarray(be)).astype(BF)[None, :]
    com["amat"] = np.asarray(mw0[0:128], np.float32).astype(BF)
    com["bmat"] = np.asarray(mw0[128:256], np.float32).astype(BF)
    com["mw0r"] = np.asarray(mw0[256], np.float32).astype(BF)[None, :]
    com["mb0r"] = np.asarray(mb0, np.float32).astype(BF)[None, :]
    com["mw1"] = np.asarray(mw1, np.float32).astype(BF)
    com["mb1r"] = np.asarray(mb1, np.float32).astype(BF)[None, :]
    com["mw2c"] = np.asarray(mw2, np.float32).astype(BF)
    com["mb2r"] = np.asarray(mb2, np.float32).reshape(1, 1).astype(BF)

    nc = _build_nc(em, cm)
    in_maps = []
    for k in range(NCORE):
        m = dict(com)
        m["xT"] = xpad[k * NSH:(k + 1) * NSH].T.astype(BF).copy()
        m["invd"] = np.tile(invdp[k * NSH:(k + 1) * NSH].astype(BF), (P, 1))
        m["gidx"] = edata[k]["gidx"]
        m["ohm"] = edata[k]["ohm"]
        m["cu"] = cdata[k]["cu"]
        m["cv"] = cdata[k]["cv"]
        m["featr"] = cdata[k]["feat"]
        m["maskr"] = cdata[k]["mask"]
        in_maps.append(m)
    trace = bool(os.environ.get("KERNEL_TRACE"))
    if trace:
        import types
        if "antenv.axon_hooks" not in sys.modules:
            try:
                import antenv
                from trn_agent_boot.trn_boot import _ntff_profile_via_ctypes
                mod = types.ModuleType("antenv.axon_hooks")
                hook = [_ntff_profile_via_ctypes("/opt/axon/libaxon_pjrt.so")]
                mod.set_axon_ntff_profile_hook = lambda h: hook.__setitem__(0, h)
                mod.get_axon_ntff_profile_hook = lambda: hook[0]
                sys.modules["antenv.axon_hooks"] = mod
                antenv.axon_hooks = mod
            except Exception:
                trace = False
    res = run_bass_kernel_spmd(nc, in_maps, core_ids=list(range(NCORE)),
                               trace=trace,
                               tmpdir=os.environ.get("KERNEL_TRACE_DIR"))
    if trace and res.exec_time_ns is not None:
        print(f"HW exec time: {res.exec_time_ns} ns")
    y_all = np.zeros(C, np.float32)
    p_all = np.zeros(C, np.float32)
    cslot = cm["cslot"]
    y_lin = res.results[0]["y_out"].ravel()   # flat order: core, slot
    p_lin = res.results[0]["p_out"].ravel()
    for k in range(NCORE):
        sm = cdata[k]["slotmap"]
        valid = sm >= 0
        j = np.nonzero(valid)[0]
        y_all[sm[valid]] = y_lin[k * cslot + j]
        p_all[sm[valid]] = p_lin[k * cslot + j]
    return y_all[:, None], p_all[:, None]


# revision 10
# speedup vs baseline: 3.0437x; 1.1396x over previous
"""Trainium2 Bass kernel for nn_PolicyNetwork3 (2-layer GraphSAGE + edge-MLP).

v2 design (8 NeuronCores, SPMD single NEFF):
- dst-sharded aggregation; core k owns node block [6272k, 6272k+6272).
- Edge messages gathered per-edge from bf16 HBM row tables via 4-queue
  round-robin dma_gather (descriptor generation parallelizes across the
  SWDGE queues; ~2.4ns/row vs 8ns serialized).
- segment-sum per 256-dst window via one-hot matmuls (bf16 gathered rows x
  fp8 0/1 one-hot streamed from HBM) accumulating in PSUM across the LO/HI
  half-table runs; per-window PSUM->SBUF copy fuses the 1/deg scale.
- BN folded into SAGE weights on host; linear+leaky per 128-node block;
  h shards exchanged with bf16 AllGather.
- candidate MLP: g/q tables per node; transposed candidate gathers put
  features on partitions so the whole MLP runs as 512-wide PE matmuls
  (identity-matmul adds, K=1 bias rows); global softmax on-device.
"""

import os
import sys

sys.path.insert(0, "/opt/trn_rl_repo")
sys.path.insert(0, "/root/.axon_site")

import numpy as np
import ml_dtypes

import concourse.bacc as bacc
import concourse.bass as bass
import concourse.bass_isa as bass_isa
import concourse.mybir as mybir
import concourse.tile as tile
from concourse import library_config
from concourse.bass_utils import run_bass_kernel_spmd

P = 128
N, E, C = 50000, 800000, 100000
D = 128
NCORE = 8
NSH = 6272            # nodes per core shard
NTOT = NSH * NCORE    # 50176 padded node table
HALF = NTOT // 2      # 25088 rows per gather-table half
W = 256               # dst window width (PSUM cols)
NWIN = (NSH + W - 1) // W   # 25 windows (last one 128 wide)
NBLK = NSH // P       # 49 node blocks for the linear phase
CSH = C // NCORE      # 12500 candidates per core
GCALL = 2048          # max idxs per dma_gather call
CCH = 512             # candidate MLP chunk
BN_EPS = 1e-5
SLOPE = 0.01
F32 = mybir.dt.float32
BF16 = mybir.dt.bfloat16
F8 = mybir.dt.float8e4
I16 = mybir.dt.int16
AF = mybir.ActivationFunctionType
ALU = mybir.AluOpType
BF = ml_dtypes.bfloat16
F8NP = ml_dtypes.float8_e4m3fn


def _wrap16(idx_lin):
    """[n] -> [128, n/16] int16 in the dma_gather wrapped+replicated layout."""
    n = idx_lin.shape[0]
    assert n % 16 == 0
    w = idx_lin.reshape(n // 16, 16).T.astype(np.int16)
    return np.tile(w, (8, 1)).copy()


def _win_width(w):
    return min(W, NSH - w * W)


def _prep_edges(src, dst):
    """Uniform per-core window/run/chunk schedule + per-core idx and one-hot."""
    core = dst // NSH
    winl = (dst - core * NSH) // W
    dstloc = (dst - core * NSH) - winl * W
    half = (src >= HALF).astype(np.int64)
    key = (core * NWIN + winl) * 2 + half
    order = np.argsort(key, kind="stable")
    cnt = np.bincount(key, minlength=NCORE * NWIN * 2).reshape(NCORE, NWIN, 2)
    nch_u = (-(-cnt // P)).max(axis=0)        # [NWIN, 2] uniform chunk counts
    # global chunk order: w0 LO chunks, w0 HI, w1 LO, ...
    win_ch0 = np.zeros(NWIN + 1, np.int64)
    np.cumsum(nch_u.sum(axis=1), out=win_ch0[1:])
    totch = int(win_ch0[-1])
    nslot = totch * P
    # gather calls (uniform): per (w, half) run split into balanced pieces
    # of ~CTGT chunks so the 4 SWDGE queues stay evenly loaded
    CTGT = 8
    calls = []  # (slot_start, n_idx, half, chunk0)
    for w in range(NWIN):
        c0 = int(win_ch0[w])
        for s in (0, 1):
            nch = int(nch_u[w, s])
            if nch > 0:
                npiece = -(-nch // CTGT)
                base, rem = divmod(nch, npiece)
                cc = c0
                for i in range(npiece):
                    sz = base + (1 if i < rem else 0)
                    calls.append((cc * P, sz * P, s, cc))
                    cc += sz
            c0 += nch
    bstart = np.zeros(NCORE * NWIN * 2 + 1, np.int64)
    np.cumsum(np.bincount(key, minlength=NCORE * NWIN * 2), out=bstart[1:])
    gidx = np.zeros((NCORE, nslot), np.int16)
    ohm = np.zeros((NCORE, P, totch * W), np.uint8)  # fp8 bits (1.0 = 0x38)
    ONE = np.float32(1.0).astype(F8NP).view(np.uint8)
    for k in range(NCORE):
        for w in range(NWIN):
            pos = int(win_ch0[w]) * P
            for s in (0, 1):
                b = (k * NWIN + w) * 2 + s
                sl = order[bstart[b]:bstart[b + 1]]
                n = len(sl)
                gidx[k, pos:pos + n] = (src[sl] - s * HALF).astype(np.int16)
                slots = pos + np.arange(n)
                ohm[k, slots % P, (slots // P) * W + dstloc[sl]] = ONE
                pos += int(nch_u[w, s]) * P
    meta = dict(nch_u=nch_u, win_ch0=win_ch0, totch=totch, nslot=nslot,
                calls=calls)
    data = [dict(gidx=_wrap16(gidx[k]), ohm=ohm[k].view(F8NP)) for k in range(NCORE)]
    return meta, data


def _prep_cands(cand_u, cand_v, cand_feat):
    """Shard candidates, group by (u_half, v_half), pad groups to 128."""
    gch = np.zeros((NCORE, 4), np.int64)
    groups = [[None] * 4 for _ in range(NCORE)]
    for k in range(NCORE):
        ids = np.arange(k * CSH, (k + 1) * CSH)
        g = (cand_u[ids] >= HALF) * 2 + (cand_v[ids] >= HALF)
        for gi in range(4):
            groups[k][gi] = ids[g == gi]
            gch[k, gi] = -(-len(groups[k][gi]) // P)
    gch_u = gch.max(axis=0)
    goff = np.zeros(5, np.int64)
    np.cumsum(gch_u * P, out=goff[1:])
    cslot = int(goff[4])                       # 128-granular
    cu = np.zeros((NCORE, cslot), np.int16)
    cv = np.zeros((NCORE, cslot), np.int16)
    ft = np.zeros((NCORE, cslot), BF)
    mask = np.full((NCORE, cslot), -1e30, np.float32).astype(BF)
    slotmap = np.full((NCORE, cslot), -1, np.int64)
    for k in range(NCORE):
        for gi in range(4):
            ids = groups[k][gi]
            n = len(ids)
            p0 = int(goff[gi])
            uh, vh = gi // 2, gi % 2
            cu[k, p0:p0 + n] = (cand_u[ids] - uh * HALF).astype(np.int16)
            cv[k, p0:p0 + n] = (cand_v[ids] - vh * HALF).astype(np.int16)
            ft[k, p0:p0 + n] = cand_feat[ids, 0].astype(BF)
            mask[k, p0:p0 + n] = 0.0
            slotmap[k, p0:p0 + n] = ids
    # u calls: groups 0-1 (uh=0) then 2-3 (uh=1); v calls per group;
    # balanced ~8-chunk pieces
    CTGT = 8

    def _split(lo, hi, s, out):
        nch = (hi - lo) // P
        if nch <= 0:
            return
        npiece = -(-nch // CTGT)
        base, rem = divmod(nch, npiece)
        p = lo
        for i in range(npiece):
            sz = (base + (1 if i < rem else 0)) * P
            out.append((p, sz, s))
            p += sz

    ucalls, vcalls = [], []
    _split(0, int(goff[2]), 0, ucalls)
    _split(int(goff[2]), int(goff[4]), 1, ucalls)
    for gi in range(4):
        _split(int(goff[gi]), int(goff[gi + 1]), gi % 2, vcalls)
    meta = dict(cslot=cslot, ucalls=ucalls, vcalls=vcalls)
    data = [dict(cu=_wrap16(cu[k]), cv=_wrap16(cv[k]), feat=ft[k][None, :],
                 mask=mask[k][None, :], slotmap=slotmap[k]) for k in range(NCORE)]
    return meta, data


def _build_nc(em, cm):
    nc = bacc.Bacc("TRN2", target_bir_lowering=False, debug=False,
                   num_devices=NCORE, num_swdge_queues=4)
    TOTCH, NSLOT = em["totch"], em["nslot"]
    CSLOT = cm["cslot"]
    NCC = -(-CSLOT // CCH)                  # candidate MLP chunk groups
    YCOLS = NCORE * CSLOT // P              # yfull viewed as [128, YCOLS]

    xb = nc.dram_tensor("xb", [NTOT, D], BF16, kind="ExternalInput")
    xT = nc.dram_tensor("xT", [P, NSH], BF16, kind="ExternalInput")
    gidx = nc.dram_tensor("gidx", [P, NSLOT // 16], I16, kind="ExternalInput")
    ohm = nc.dram_tensor("ohm", [P, TOTCH * W], F8, kind="ExternalInput")
    invd = nc.dram_tensor("invd", [P, NSH], BF16, kind="ExternalInput")
    wself = [nc.dram_tensor(f"wself{l}", [D, D], BF16, kind="ExternalInput") for l in range(2)]
    wneigh = [nc.dram_tensor(f"wneigh{l}", [D, D], BF16, kind="ExternalInput") for l in range(2)]
    crow = [nc.dram_tensor(f"crow{l}", [1, D], BF16, kind="ExternalInput") for l in range(2)]
    identb = nc.dram_tensor("identb", [P, P], BF16, kind="ExternalInput")
    onesr = nc.dram_tensor("onesr", [1, P], BF16, kind="ExternalInput")
    onesc = nc.dram_tensor("onesc", [1, CCH], BF16, kind="ExternalInput")
    amat = nc.dram_tensor("amat", [D, 64], BF16, kind="ExternalInput")
    bmat = nc.dram_tensor("bmat", [D, 64], BF16, kind="ExternalInput")
    mw0r = nc.dram_tensor("mw0r", [1, 64], BF16, kind="ExternalInput")
    mb0r = nc.dram_tensor("mb0r", [1, 64], BF16, kind="ExternalInput")
    mw1 = nc.dram_tensor("mw1", [64, 64], BF16, kind="ExternalInput")
    mb1r = nc.dram_tensor("mb1r", [1, 64], BF16, kind="ExternalInput")
    mw2 = nc.dram_tensor("mw2c", [64, 1], BF16, kind="ExternalInput")
    mb2r = nc.dram_tensor("mb2r", [1, 1], BF16, kind="ExternalInput")
    cu = nc.dram_tensor("cu", [P, CSLOT // 16], I16, kind="ExternalInput")
    cv = nc.dram_tensor("cv", [P, CSLOT // 16], I16, kind="ExternalInput")
    featr = nc.dram_tensor("featr", [1, CSLOT], BF16, kind="ExternalInput")
    maskr = nc.dram_tensor("maskr", [1, CSLOT], BF16, kind="ExternalInput")

    y_out = nc.dram_tensor("y_out", [P, YCOLS], F32, kind="ExternalOutput")
    p_out = nc.dram_tensor("p_out", [P, YCOLS], F32, kind="ExternalOutput")

    hsh = nc.dram_tensor("hsh", [NSH, D], BF16, kind="Internal")
    hfull = nc.dram_tensor("hfull", [NTOT, D], BF16, kind="Internal",
                           addr_space="Shared")
    gqsh = nc.dram_tensor("gqsh", [NSH, D], BF16, kind="Internal")
    gqfull = nc.dram_tensor("gqfull", [NTOT, D], BF16, kind="Internal",
                            addr_space="Shared")
    ysh = nc.dram_tensor("ysh", [1, CSLOT], F32, kind="Internal")
    yfull = nc.dram_tensor("yfull", [NCORE, CSLOT], F32, kind="Internal",
                           addr_space="Shared")

    rg = [list(range(NCORE))]
    nch_u, win_ch0, calls = em["nch_u"], em["win_ch0"], em["calls"]

    with tile.TileContext(nc) as tc:
        with (
            tc.tile_pool(name="const", bufs=1) as cp,
            tc.tile_pool(name="big", bufs=1) as bp,
            tc.tile_pool(name="msgs", bufs=10) as mp,
            tc.tile_pool(name="ohp", bufs=4) as op_,
            tc.tile_pool(name="wrk", bufs=4) as wp,
            tc.tile_pool(name="frows", bufs=3) as fp_,
            tc.tile_pool(name="gup", bufs=3) as gup,
            tc.tile_pool(name="gvp", bufs=3) as gvp,
            tc.tile_pool(name="z1p", bufs=2) as z1p,
        ):
            nc.gpsimd.load_library(library_config.mlp)

            def load(pool, t, shape=None):
                tl = pool.tile(shape or list(t.shape), t.dtype, tag=t.name)
                nc.sync.dma_start(tl[:], t[:])
                return tl

            gidx_t = load(cp, gidx)
            invd_t = load(cp, invd)
            identb_t = load(cp, identb)
            onesr_t = load(cp, onesr)
            onesc_t = load(cp, onesc)
            wself_t = [load(cp, t) for t in wself]
            wneigh_t = [load(cp, t) for t in wneigh]
            crow_t = [load(cp, t) for t in crow]
            amat_t = load(cp, amat)
            bmat_t = load(cp, bmat)
            mw0r_t = load(cp, mw0r)
            mb0r_t = load(cp, mb0r)
            mw1_t = load(cp, mw1)
            mb1r_t = load(cp, mb1r)
            mw2_t = load(cp, mw2)
            mb2r_t = load(cp, mb2r)
            cu_t = load(cp, cu)
            cv_t = load(cp, cv)
            featr_t = load(cp, featr)

            curT = bp.tile([P, NSH], BF16, tag="curT")
            nxtT = bp.tile([P, NSH], BF16, tag="nxtT")
            aggrb = bp.tile([P, NSH], BF16, tag="aggrb")
            nc.sync.dma_start(curT[:], xT[:])

            qrr = [0]

            def next_q():
                q = qrr[0]
                qrr[0] = (q + 1) % 4
                return q

            ctxA = tc.tile_pool(name="ps_w", bufs=2, space="PSUM")
            ps_w = ctxA.__enter__()
            ctxB = tc.tile_pool(name="ps_l", bufs=2, space="PSUM")
            ps_l = ctxB.__enter__()
            ctxC = tc.tile_pool(name="ps_t", bufs=1, space="PSUM")
            ps_t = ctxC.__enter__()

            for layer in range(2):
                gtab = xb if layer == 0 else hfull
                # window state: psum tile + chunks left
                win_ps = {}
                ci = 0  # call index cursor (calls are in window order)
                for w in range(NWIN):
                    ww = _win_width(w)
                    c0 = int(win_ch0[w])
                    mtot = int(nch_u[w, 0] + nch_u[w, 1])
                    if mtot == 0:
                        nc.vector.memset(aggrb[:, w * W:w * W + ww], 0.0)
                        continue
                    ps = ps_w.tile([P, W], F32, tag="psw")
                    done = 0
                    for s in (0, 1):
                        nch = int(nch_u[w, s])
                        left = nch
                        while left > 0:
                            (p0, n_idx, sh, ch0) = calls[ci]
                            ci += 1
                            ncall = n_idx // P
                            g = mp.tile([P, GCALL // P, P], BF16, tag="g")
                            nc.gpsimd.dma_gather(
                                g[:, :ncall, :],
                                gtab[sh * HALF:(sh + 1) * HALF, :],
                                gidx_t[:, p0 // 16:(p0 + n_idx) // 16],
                                n_idx, n_idx, P, single_packet=False,
                                queue_num=next_q())
                            oht = op_.tile([P, GCALL // P, W], F8, tag="oh")
                            nc.sync.dma_start(
                                oht[:, :ncall, :],
                                ohm[:, ch0 * W:(ch0 + ncall) * W]
                                .rearrange("p (a b) -> p a b", b=W))
                            for cc in range(ncall):
                                nc.tensor.matmul(
                                    ps[:], lhsT=g[:, cc, :], rhs=oht[:, cc, :],
                                    start=(done == 0), stop=(done == mtot - 1))
                                done += 1
                            left -= ncall
                    # fused PSUM->SBUF copy with 1/deg scale
                    nc.vector.tensor_tensor(
                        aggrb[:, w * W:w * W + ww], ps[:, :ww],
                        invd_t[:, w * W:w * W + ww], ALU.mult)
                    # linear phase for completed 128-node blocks
                    for b in range((w * W) // P, (w * W + ww) // P):
                        ph = ps_l.tile([P, P], F32, tag="ph")
                        nc.tensor.matmul(ph[:], lhsT=aggrb[:, b * P:(b + 1) * P],
                                         rhs=wneigh_t[layer][:],
                                         start=True, stop=False)
                        nc.tensor.matmul(ph[:], lhsT=curT[:, b * P:(b + 1) * P],
                                         rhs=wself_t[layer][:],
                                         start=False, stop=False)
                        nc.tensor.matmul(ph[:], lhsT=onesr_t[:],
                                         rhs=crow_t[layer][:],
                                         start=False, stop=True)
                        tmp = wp.tile([P, P], BF16, tag="tmp")
                        nc.vector.tensor_scalar(tmp[:], ph[:], SLOPE, None,
                                                ALU.mult)
                        ht = wp.tile([P, P], BF16, tag="ht")
                        nc.vector.tensor_tensor(ht[:], ph[:], tmp[:], ALU.max)
                        if layer == 0:
                            nc.sync.dma_start(hsh[b * P:(b + 1) * P, :], ht[:])
                        pt = ps_t.tile([P, P], BF16, tag="pt")
                        nc.tensor.transpose(pt[:], ht[:], identb_t[:])
                        nc.scalar.activation(nxtT[:, b * P:(b + 1) * P], pt[:],
                                             AF.Copy)
                if layer == 0:
                    nc.gpsimd.collective_compute(
                        "AllGather", ALU.bypass, replica_groups=rg,
                        ins=[hsh[:].opt()], outs=[hfull[:].opt()])
                curT, nxtT = nxtT, curT

            # ---- g/q tables (cols 0:64 g = h2@A, 64:128 q = h2@B + mb0) ----
            for b in range(NBLK):
                pg = ps_l.tile([P, P], F32, tag="ph")
                nc.tensor.matmul(pg[:, 0:64], lhsT=curT[:, b * P:(b + 1) * P],
                                 rhs=amat_t[:], start=True, stop=True)
                nc.tensor.matmul(pg[:, 64:128], lhsT=curT[:, b * P:(b + 1) * P],
                                 rhs=bmat_t[:], start=True, stop=False)
                nc.tensor.matmul(pg[:, 64:128], lhsT=onesr_t[:], rhs=mb0r_t[:],
                                 start=False, stop=True)
                gq = wp.tile([P, P], BF16, tag="tmp")
                nc.scalar.activation(gq[:], pg[:], AF.Copy)
                nc.sync.dma_start(gqsh[b * P:(b + 1) * P, :], gq[:])
            nc.gpsimd.collective_compute(
                "AllGather", ALU.bypass, replica_groups=rg,
                ins=[gqsh[:].opt()], outs=[gqfull[:].opt()])

            ctxC.__exit__(None, None, None)
            ctxB.__exit__(None, None, None)
            ctxA.__exit__(None, None, None)
            with (
                tc.tile_pool(name="psc1", bufs=2, space="PSUM") as psc1,
                tc.tile_pool(name="psc2", bufs=2, space="PSUM") as psc2,
                tc.tile_pool(name="ps_c", bufs=2, space="PSUM") as ps_c,
                tc.tile_pool(name="ps_y", bufs=1, space="PSUM") as ps_y,
            ):
                # ---- candidate gathers (slot layout, 4-queue) ----
                ucalls, vcalls = cm["ucalls"], cm["vcalls"]
                uci, vci = [], []
                for i, (s0, n, sh) in enumerate(ucalls):
                    uci += [(i, j) for j in range(n // P)]
                for i, (s0, n, sh) in enumerate(vcalls):
                    vci += [(i, j) for j in range(n // P)]
                gu_tiles = [None] * len(ucalls)
                gv_tiles = [None] * len(vcalls)
                for i in range(max(len(ucalls), len(vcalls))):
                    for (tiles, calls, pool, idx_t) in (
                            (gu_tiles, ucalls, gup, cu_t),
                            (gv_tiles, vcalls, gvp, cv_t)):
                        if i >= len(calls):
                            continue
                        (s0, n_idx, sh) = calls[i]
                        ncall = n_idx // P
                        g = pool.tile([P, GCALL // P, P], BF16, tag="g")
                        nc.gpsimd.dma_gather(
                            g[:, :ncall, :],
                            gqfull[sh * HALF:(sh + 1) * HALF, :],
                            idx_t[:, s0 // 16:(s0 + n_idx) // 16],
                            n_idx, n_idx, P, single_packet=False,
                            queue_num=next_q())
                        tiles[i] = g
                # ---- candidate MLP ----
                nchunk = CSLOT // P
                z1t = None
                for c in range(nchunk):
                    ui, uo = uci[c]
                    vi, vo = vci[c]
                    psf = psc1.tile([P, 64], F32, tag="psf")
                    nc.tensor.matmul(psf[:], lhsT=identb_t[:],
                                     rhs=gu_tiles[ui][:, uo, 0:64],
                                     start=True, stop=False)
                    nc.tensor.matmul(psf[:], lhsT=identb_t[:],
                                     rhs=gv_tiles[vi][:, vo, 64:128],
                                     start=False, stop=False)
                    nc.tensor.matmul(psf[:], lhsT=featr_t[:, c * P:(c + 1) * P],
                                     rhs=mw0r_t[:], start=False, stop=True)
                    zs = wp.tile([P, 64], BF16, tag="zs")
                    nc.vector.tensor_scalar(zs[:], psf[:], SLOPE, None, ALU.mult)
                    z1b = wp.tile([P, 64], BF16, tag="z1b")
                    nc.vector.tensor_tensor(z1b[:], psf[:], zs[:], ALU.max)
                    ptz = psc2.tile([64, P], BF16, tag="ptz")
                    nc.tensor.transpose(ptz[:], z1b[:], identb_t[:])
                    if c % 4 == 0:
                        z1t = z1p.tile([64, CCH], BF16, tag="z1t")
                    nc.scalar.activation(z1t[:, (c % 4) * P:(c % 4 + 1) * P],
                                         ptz[:], AF.Copy)
                    if c % 4 == 3 or c == nchunk - 1:
                        g0 = (c // 4) * CCH
                        gw = min(CCH, CSLOT - g0)
                        mr = fp_.tile([1, CCH], BF16, tag="mr")
                        nc.sync.dma_start(mr[:, :gw], maskr[:, g0:g0 + gw])
                        psz = ps_c.tile([64, CCH], F32, tag="psc")
                        nc.tensor.matmul(psz[:, :gw], lhsT=mw1_t[:],
                                         rhs=z1t[:, :gw], start=True, stop=False)
                        nc.tensor.matmul(psz[:, :gw], lhsT=mb1r_t[:],
                                         rhs=onesc_t[:, :gw],
                                         start=False, stop=True)
                        zs2 = wp.tile([64, CCH], BF16, tag="zs2")
                        nc.vector.tensor_scalar(zs2[:, :gw], psz[:, :gw],
                                                SLOPE, None, ALU.mult)
                        z2 = wp.tile([64, CCH], BF16, tag="z2")
                        nc.vector.tensor_tensor(z2[:, :gw], psz[:, :gw],
                                                zs2[:, :gw], ALU.max)
                        py = ps_y.tile([1, CCH], F32, tag="psy")
                        nc.tensor.matmul(py[:, :gw], lhsT=mw2_t[:],
                                         rhs=z2[:, :gw], start=True, stop=False)
                        nc.tensor.matmul(py[:, :gw], lhsT=mb2r_t[:],
                                         rhs=onesc_t[:, :gw],
                                         start=False, stop=True)
                        ym = fp_.tile([1, CCH], F32, tag="ym")
                        nc.vector.tensor_tensor(ym[:, :gw], py[:, :gw],
                                                mr[:, :gw], ALU.add)
                        nc.sync.dma_start(ysh[:, g0:g0 + gw], ym[:, :gw])
            nc.gpsimd.collective_compute(
                "AllGather", ALU.bypass, replica_groups=rg,
                ins=[ysh[:].opt()], outs=[yfull[:].opt()])
            # ---- softmax over yfull viewed as [128, YCOLS] ----
            yf = bp.tile([P, YCOLS], F32, tag="yf")
            nc.sync.dma_start(yf[:], yfull[:].rearrange("a b -> (a b)")
                              .rearrange("(p c) -> p c", p=P))
            nc.sync.dma_start(y_out[:], yf[:])
            rmax = wp.tile([P, 1], F32, tag="rmax")
            nc.vector.tensor_reduce(rmax[:], yf[:], mybir.AxisListType.X, ALU.max)
            gmax = wp.tile([P, 1], F32, tag="gmax")
            nc.gpsimd.partition_all_reduce(gmax[:], rmax[:], P,
                                           bass_isa.ReduceOp.max)
            ngmax = wp.tile([P, 1], F32, tag="ngmax")
            nc.vector.tensor_scalar(ngmax[:], gmax[:], -1.0, None, ALU.mult)
            ef = bp.tile([P, YCOLS], F32, tag="ef")
            se = wp.tile([P, 1], F32, tag="se")
            nc.scalar.activation(ef[:], yf[:], AF.Exp, bias=ngmax[:, 0:1],
                                 accum_out=se[:])
            stot = wp.tile([P, 1], F32, tag="stot")
            nc.gpsimd.partition_all_reduce(stot[:], se[:], P,
                                           bass_isa.ReduceOp.add)
            invs = wp.tile([P, 1], F32, tag="invs")
            nc.vector.reciprocal(invs[:], stot[:])
            pf = bp.tile([P, YCOLS], F32, tag="pf")
            nc.vector.tensor_scalar(pf[:], ef[:], invs[:, 0:1], None, ALU.mult)
            nc.sync.dma_start(p_out[:], pf[:])
    nc.compile()
    return nc


def kernel(x, src, dst, cand_u, cand_v, cand_feat,
           w_self0, w_neigh0, b0, gamma0, beta0, rm0, rv0,
           w_self1, w_neigh1, b1, gamma1, beta1, rm1, rv1,
           mw0, mb0, mw1, mb1, mw2, mb2):
    x = np.asarray(x, np.float32)
    src = np.asarray(src, np.int64)
    dst = np.asarray(dst, np.int64)
    cand_u = np.asarray(cand_u, np.int64)
    cand_v = np.asarray(cand_v, np.int64)
    cand_feat = np.asarray(cand_feat, np.float32)

    deg = np.bincount(dst, minlength=N).astype(np.float32)
    invdeg = 1.0 / np.maximum(deg, 1.0)
    em, edata = _prep_edges(src, dst)
    cm, cdata = _prep_cands(cand_u, cand_v, cand_feat)

    xpad = np.zeros((NTOT, D), np.float32)
    xpad[:N] = x
    invdp = np.zeros(NTOT, np.float32)
    invdp[:N] = invdeg

    com = {
        "xb": xpad.astype(BF),
        "identb": np.eye(P, dtype=np.float32).astype(BF),
        "onesr": np.ones((1, P), BF),
        "onesc": np.ones((1, CCH), BF),
    }
    for l, (ws, wn, b, ga, be, rme, rve) in enumerate(
        ((w_self0, w_neigh0, b0, gamma0, beta0, rm0, rv0),
         (w_self1, w_neigh1, b1, gamma1, beta1, rm1, rv1))):
        a = (np.asarray(ga) / np.sqrt(np.asarray(rve) + BN_EPS)).astype(np.float32)
        com[f"wself{l}"] = (np.asarray(ws) * a[None, :]).astype(BF)
        com[f"wneigh{l}"] = (np.asarray(wn) * a[None, :]).astype(BF)
        com[f"crow{l}"] = (a * (np.asarray(b) - np.asarray(rme)) + np.asarray(be)).astype(BF)[None, :]
    com["amat"] = np.asarray(mw0[0:128], np.float32).astype(BF)
    com["bmat"] = np.asarray(mw0[128:256], np.float32).astype(BF)
    com["mw0r"] = np.asarray(mw0[256], np.float32).astype(BF)[None, :]
    com["mb0r"] = np.asarray(mb0, np.float32).astype(BF)[None, :]
    com["mw1"] = np.asarray(mw1, np.float32).astype(BF)
    com["mb1r"] = np.asarray(mb1, np.float32).astype(BF)[None, :]
    com["mw2c"] = np.asarray(mw2, np.float32).astype(BF)
    com["mb2r"] = np.asarray(mb2, np.float32).reshape(1, 1).astype(BF)

    nc = _build_nc(em, cm)
    in_maps = []
    for k in range(NCORE):
        m = dict(com)
        m["xT"] = xpad[k * NSH:(k + 1) * NSH].T.astype(BF).copy()
        m["invd"] = np.tile(invdp[k * NSH:(k + 1) * NSH].astype(BF), (P, 1))
        m["gidx"] = edata[k]["gidx"]
        m["ohm"] = edata[k]["ohm"]
        m["cu"] = cdata[k]["cu"]
        m["cv"] = cdata[k]["cv"]
        m["featr"] = cdata[k]["feat"]
        m["maskr"] = cdata[k]["mask"]
        in_maps.append(m)
    trace = bool(os.environ.get("KERNEL_TRACE"))
    if trace:
        import types
        if "antenv.axon_hooks" not in sys.modules:
            try:
                import antenv
                from trn_agent_boot.trn_boot import _ntff_profile_via_ctypes
                mod = types.ModuleType("antenv.axon_hooks")
                hook = [_ntff_profile_via_ctypes("/opt/axon/libaxon_pjrt.so")]
                mod.set_axon_ntff_profile_hook = lambda h: hook.__setitem__(0, h)
                mod.get_axon_ntff_profile_hook = lambda: hook[0]
                sys.modules["antenv.axon_hooks"] = mod
                antenv.axon_hooks = mod
            except Exception:
                trace = False
    res = run_bass_kernel_spmd(nc, in_maps, core_ids=list(range(NCORE)),
                               trace=trace,
                               tmpdir=os.environ.get("KERNEL_TRACE_DIR"))
    if trace and res.exec_time_ns is not None:
        print(f"HW exec time: {res.exec_time_ns} ns")
    y_all = np.zeros(C, np.float32)
    p_all = np.zeros(C, np.float32)
    cslot = cm["cslot"]
    y_lin = res.results[0]["y_out"].ravel()   # flat order: core, slot
    p_lin = res.results[0]["p_out"].ravel()
    for k in range(NCORE):
        sm = cdata[k]["slotmap"]
        valid = sm >= 0
        j = np.nonzero(valid)[0]
        y_all[sm[valid]] = y_lin[k * cslot + j]
        p_all[sm[valid]] = p_lin[k * cslot + j]
    return y_all[:, None], p_all[:, None]
